# revision 1
# baseline (speedup 1.0000x reference)
"""2-layer GATv2 + global mean pool + linear head, on 8 Trainium2 NeuronCores.

Strategy (dst-sharded, degree-balanced static schedule):
  - Nodes are relabeled by a degree-balanced bin-packing into groups of <=128
    destination nodes such that every group has nearly equal incident-edge
    count.  This makes the per-group chunk count S uniform, so one SPMD
    program works for all 8 cores.
  - Core c owns nodes [c*own, (c+1)*own) (new ids).  Edges (with self loops)
    are assigned by destination owner, laid out as groups x S chunks of 128
    edge slots.
  - Per chunk: indirect-DMA gather of xl[src] and xr[dst] rows (bf16),
    z = G + V, leaky-relu, per-head score = reduce(l * att), p = exp(score),
    indicator = is_equal(dstloc, iota) and one PE matmul accumulates both
    the weighted feature sums and the softmax denominators into PSUM.
  - Per group: divide by denominators, add bias, ELU -> h row block.
  - Layer tables xl = x@W_l + b_l are built on device; the layer-2 source
    table is AllGather'ed across cores.  Final pooling partial sums are
    AllReduce'd and every core computes the tiny output head.
"""

import sys

for _p in ("/opt/trn_rl_repo",):
    if _p not in sys.path:
        sys.path.insert(0, _p)

import numpy as np
import ml_dtypes

BF = ml_dtypes.bfloat16

import concourse.bass as bass
import concourse.mybir as mybir
from concourse.tile import TileContext
from concourse.bass_utils import run_bass_kernel_spmd
from concourse.masks import make_identity

F32 = mybir.dt.float32
BF16 = mybir.dt.bfloat16
I32 = mybir.dt.int32
P = 128
NCORES = 8
NUM_GRAPHS = 64
NEG_SLOPE = 0.2


# ---------------------------------------------------------------- prof hook
def _install_profhook():
    """Provide antenv.axon_hooks (absent in this image) so trace=True works."""
    import types

    if "antenv.axon_hooks" in sys.modules:
        return
    try:
        from trn_agent_boot.trn_boot import _ntff_profile_via_ctypes
    except Exception:
        return
    mod = types.ModuleType("antenv.axon_hooks")
    mod._hook = None
    mod.set_axon_ntff_profile_hook = lambda h: setattr(mod, "_hook", h)
    mod.get_axon_ntff_profile_hook = lambda: mod._hook
    sys.modules["antenv.axon_hooks"] = mod
    try:
        mod._hook = _ntff_profile_via_ctypes("/opt/axon/libaxon_pjrt.so")
    except Exception:
        mod._hook = None


# ---------------------------------------------------------------- wait split
def _split_waits(nc, max_waits=1):
    """walrus TPB_CTRL codegen rejects >1 sync-wait per instruction; move
    extras onto preceding NoOps on the same engine."""
    n_added = 0
    for fn in nc.m.functions:
        for blk in fn.blocks:
            new_insts = []
            for inst in blk.instructions:
                si = getattr(inst, "sync_info", None)
                waits = list(si.on_wait) if si is not None and si.on_wait else []
                if len(waits) > max_waits:
                    extra = waits[:-max_waits]
                    for i in range(0, len(extra), max_waits):
                        chunk = extra[i : i + max_waits]
                        nop = mybir.InstNoOp(
                            name=f"{inst.name}_wsplit{n_added}",
                            engine=inst.engine,
                            ins=[],
                            outs=[],
                            sync_info=mybir.SyncInfo(on_wait=chunk, on_update=[]),
                        )
                        n_added += 1
                        new_insts.append(nop)
                    si.on_wait = waits[-max_waits:]
                new_insts.append(inst)
            blk.instructions = new_insts
    return n_added


# ---------------------------------------------------------------- host prep
def _prep(x, edge_index, batch, ncores):
    """Degree-balanced relabeling + per-core static edge layout."""
    N = x.shape[0]
    own = N // ncores
    gfull, rem = divmod(own, P)
    ngroups = gfull + (1 if rem else 0)

    src = np.concatenate([edge_index[0].astype(np.int64), np.arange(N)])
    dst = np.concatenate([edge_index[1].astype(np.int64), np.arange(N)])
    deg = np.bincount(dst, minlength=N)

    # bins: per core, gfull bins of cap P then (if rem) one bin of cap rem
    caps = []
    for c in range(ncores):
        caps += [P] * gfull + ([rem] if rem else [])
    nbins = len(caps)

    import heapq

    heap = [(0, b) for b in range(nbins)]
    heapq.heapify(heap)
    bin_nodes = [[] for _ in range(nbins)]
    order = np.argsort(-deg, kind="stable")
    for node in order:
        while True:
            s, b = heapq.heappop(heap)
            if len(bin_nodes[b]) < caps[b]:
                break
        bin_nodes[b].append(node)
        if len(bin_nodes[b]) < caps[b]:
            heapq.heappush(heap, (s + int(deg[node]), b))

    perm = np.empty(N, np.int64)  # perm[new] = old
    for b in range(nbins):
        c, g = divmod(b, ngroups)
        base = c * own + g * P
        nodes = bin_nodes[b]
        perm[base : base + len(nodes)] = nodes
    inv = np.empty(N, np.int64)
    inv[perm] = np.arange(N)

    bin_sums = np.array([deg[bin_nodes[b]].sum() for b in range(nbins)])
    S = int(np.ceil(bin_sums.max() / P))
    nchunk = ngroups * S

    new_src = inv[src]
    new_dst = inv[dst]
    core_of = new_dst // own

    per_core = []
    for c in range(ncores):
        m = core_of == c
        es = new_src[m]
        ed = new_dst[m] - c * own  # own-local
        eg = np.minimum(ed // P, ngroups - 1)
        eslot = ed - eg * P
        o = np.lexsort((eslot, eg))
        es, ed, eg, eslot = es[o], ed[o], eg[o], eslot[o]

        counts = np.bincount(eg, minlength=ngroups)
        assert counts.max() <= S * P, (counts.max(), S * P)
        gstart = np.zeros(ngroups, np.int64)
        gstart[1:] = np.cumsum(counts)[:-1]
        jw = np.arange(len(es)) - gstart[eg]
        fpos = eg * S * P + jw

        flat_src = np.zeros(nchunk * P, np.int32)
        flat_dst = np.zeros(nchunk * P, np.int32)
        flat_loc = np.full(nchunk * P, 255.0, np.float32)
        flat_src[fpos] = es
        flat_dst[fpos] = ed
        flat_loc[fpos] = eslot

        srcidx = flat_src.reshape(nchunk, P).T.copy()
        dstidx = flat_dst.reshape(nchunk, P).T.copy()
        dstloc = flat_loc.reshape(nchunk, P).T.astype(BF)

        xT_own = np.ascontiguousarray(x[perm[c * own : (c + 1) * own]].T).astype(BF)

        bl = np.full((P, ngroups), 255.0, np.float32)
        for g in range(ngroups):
            size = P if (g < gfull or rem == 0) else rem
            ids = perm[c * own + g * P : c * own + g * P + size]
            bl[:size, g] = batch[ids]
        batchloc = bl.astype(BF)

        per_core.append(
            dict(
                srcidx=srcidx,
                dstidx=dstidx,
                dstloc=dstloc,
                xT_own=xT_own,
                batchloc=batchloc,
            )
        )

    meta = dict(
        N=N,
        own=own,
        ngroups=ngroups,
        gfull=gfull,
        rem=rem,
        S=S,
        nchunk=nchunk,
        ncores=ncores,
    )
    return per_core, meta, perm


# ---------------------------------------------------------------- kernel build
def _gsize(meta, g):
    return P if (g < meta["gfull"] or meta["rem"] == 0) else meta["rem"]


def _build(meta, heads1=8, heads2=1, debug=False):
    N = meta["N"]
    own = meta["own"]
    ngroups = meta["ngroups"]
    S = meta["S"]
    nchunk = meta["nchunk"]
    ncores = meta["ncores"]
    D = 128

    nc = bass.Bass(target_bir_lowering=False, debug=True)

    # ---- external inputs (per core)
    xT_in = nc.declare_dram_parameter("xT_own", [P, own], BF16, isOutput=False)
    srcidx_in = nc.declare_dram_parameter("srcidx", [P, nchunk], I32, isOutput=False)
    dstidx_in = nc.declare_dram_parameter("dstidx", [P, nchunk], I32, isOutput=False)
    dstloc_in = nc.declare_dram_parameter("dstloc", [P, nchunk], BF16, isOutput=False)
    batchloc_in = nc.declare_dram_parameter(
        "batchloc", [P, ngroups], BF16, isOutput=False
    )
    # consts (replicated)
    wnames = [
        ("W1_l", [P, D]), ("W1_r", [P, D]), ("W2_l", [P, D]), ("W2_r", [P, D]),
        ("att1_rep", [P, D]), ("att2_rep", [P, D]),
        ("bias1_rep", [P, D]), ("bias2_rep", [P, D]),
        ("b1_l", [1, D]), ("b1_r", [1, D]), ("b2_l", [1, D]), ("b2_r", [1, D]),
        ("iota128", [P, P]), ("iota64", [P, NUM_GRAPHS]),
        ("W3", [P, 10]), ("b3row", [1, 10]), ("ones1", [1, P]),
    ]
    w_in = {n: nc.declare_dram_parameter(n, sh, BF16, isOutput=False) for n, sh in wnames}
    out_t = nc.declare_dram_parameter("out", [NUM_GRAPHS, 10], F32, isOutput=True)
    dbg = {}
    if debug:
        for n, sh in [("dbg_xl1", [own, D]), ("dbg_xr1", [own, D]),
                      ("dbg_xl1full", [own * ncores, D]), ("dbg_h1T", [P, own]),
                      ("dbg_xl2", [own, D]), ("dbg_pool", [NUM_GRAPHS, 129])]:
            dt = F32 if n == "dbg_pool" else BF16
            dbg[n] = nc.declare_dram_parameter(n, sh, dt, isOutput=True)
        dbg["dbg_red"] = nc.declare_dram_parameter(
            "dbg_red", [NUM_GRAPHS, 129], F32, isOutput=True)

    # ---- internal DRAM
    xl1_own = nc.dram_tensor("xl1_own", [own, D], BF16)
    xr1_own = nc.dram_tensor("xr1_own", [own, D], BF16)
    xl1_full = nc.dram_tensor("xl1_full", [own * ncores, D], BF16, addr_space="Shared")
    xl2_own = nc.dram_tensor("xl2_own", [own, D], BF16)
    xr2_own = nc.dram_tensor("xr2_own", [own, D], BF16)
    xl2_full = nc.dram_tensor("xl2_full", [own * ncores, D], BF16, addr_space="Shared")
    pool_stage = nc.dram_tensor("pool_stage", [NUM_GRAPHS, 129], F32)
    pool_red = nc.dram_tensor("pool_red", [NUM_GRAPHS, 129], F32, addr_space="Shared")

    # ---- persistent SBUF
    sb = {}
    def persist(name, shape, dtype):
        sb[name] = nc.alloc_sbuf_tensor(name, shape, dtype)
        return sb[name]

    xT_sb = persist("xT_sb", [P, own], BF16)
    srcidx_sb = persist("srcidx_sb", [P, nchunk], I32)
    dstidx_sb = persist("dstidx_sb", [P, nchunk], I32)
    dstloc_sb = persist("dstloc_sb", [P, nchunk], BF16)
    batchloc_sb = persist("batchloc_sb", [P, ngroups], BF16)
    h1T_sb = persist("h1T_sb", [P, own], BF16)
    ident_sb = persist("ident_sb", [P, P], BF16)
    w_sb = {n: persist(n + "_sb", sh, BF16) for n, sh in wnames}

    def collective(kind, op, ins, outs):
        nc.gpsimd.collective_compute(
            kind, op, replica_groups=[list(range(ncores))], ins=ins, outs=outs
        )

    # ================= TC-load: all constant loads =================
    with TileContext(nc) as tc:
        nc.sync.dma_start(out=xT_sb[:], in_=xT_in[:])
        nc.sync.dma_start(out=srcidx_sb[:], in_=srcidx_in[:])
        nc.sync.dma_start(out=dstidx_sb[:], in_=dstidx_in[:])
        nc.sync.dma_start(out=dstloc_sb[:], in_=dstloc_in[:])
        nc.sync.dma_start(out=batchloc_sb[:], in_=batchloc_in[:])
        for n, _sh in wnames:
            nc.sync.dma_start(out=w_sb[n][:], in_=w_in[n][:])
        with tc.tile_pool(name="idp", bufs=1) as idp:
            idt = idp.tile([P, P], F32)
            make_identity(nc, idt[:])
            nc.vector.tensor_copy(out=ident_sb[:], in_=idt[:])

    # ================= TC0: build xl1/xr1 tables =================
    def build_tables(tc, srcT_sb, Wl, Wr, bl, br, out_l, out_r):
        with (
            tc.tile_pool(name="tp", bufs=3) as tp,
            tc.tile_pool(name="tpp", bufs=3, space="PSUM") as tpp,
        ):
            for g in range(ngroups):
                w = _gsize(meta, g)
                for W, brow, dest in ((Wl, bl, out_l), (Wr, br, out_r)):
                    ps = tpp.tile([P, D], F32, tag="ps")
                    nc.tensor.matmul(
                        out=ps[:w, :],
                        lhsT=srcT_sb[:, g * P : g * P + w],
                        rhs=w_sb[W][:],
                        start=True,
                        stop=False,
                    )
                    nc.tensor.matmul(
                        out=ps[:w, :],
                        lhsT=w_sb["ones1"][:, :w],
                        rhs=w_sb[brow][:],
                        start=False,
                        stop=True,
                    )
                    ot = tp.tile([P, D], BF16, tag="ot")
                    nc.scalar.activation(
                        out=ot[:w, :], in_=ps[:w, :],
                        func=mybir.ActivationFunctionType.Copy,
                    )
                    nc.sync.dma_start(
                        out=dest[g * P : g * P + w, :], in_=ot[:w, :]
                    )

    with TileContext(nc) as tc:
        build_tables(tc, xT_sb, "W1_l", "W1_r", "b1_l", "b1_r", xl1_own, xr1_own)

    # ================= edge layer =================
    def edge_layer(tc, xl_full_t, xr_own_t, att_rep, bias_rep, heads, pool_ctx=None):
        """one GATv2 layer over the static edge schedule.
        pool_ctx: None for layer1 (writes h1T_sb); else (pool_psum,) for layer2."""
        C = D // heads
        NH = heads
        with (
            tc.tile_pool(name="gv", bufs=6) as gv,
            tc.tile_pool(name="work", bufs=3) as work,
            tc.tile_pool(name="rhsp", bufs=3) as rhsp,
            tc.tile_pool(name="sc", bufs=3) as scp,
            tc.tile_pool(name="ep", bufs=2) as ep,
            tc.tile_pool(name="aggp", bufs=2, space="PSUM") as aggp,
            tc.tile_pool(name="tpsum", bufs=2, space="PSUM") as tpsum,
        ):
            for g in range(ngroups):
                w = _gsize(meta, g)
                agg = aggp.tile([P, D + NH], F32, tag="agg")
                for j in range(S):
                    k = g * S + j
                    G = gv.tile([P, D], BF16, tag="G")
                    V = gv.tile([P, D], BF16, tag="V")
                    nc.gpsimd.indirect_dma_start(
                        out=G[:], out_offset=None, in_=xl_full_t[:],
                        in_offset=bass.IndirectOffsetOnAxis(
                            ap=srcidx_sb[:, k : k + 1], axis=0),
                    )
                    nc.gpsimd.indirect_dma_start(
                        out=V[:], out_offset=None, in_=xr_own_t[:],
                        in_offset=bass.IndirectOffsetOnAxis(
                            ap=dstidx_sb[:, k : k + 1], axis=0),
                    )
                    z = work.tile([P, D], BF16, tag="z")
                    nc.vector.tensor_tensor(out=z[:], in0=G[:], in1=V[:],
                                            op=mybir.AluOpType.add)
                    zs = work.tile([P, D], BF16, tag="zs")
                    nc.vector.tensor_scalar(out=zs[:], in0=z[:],
                                            scalar1=NEG_SLOPE, scalar2=None,
                                            op0=mybir.AluOpType.mult)
                    lr = work.tile([P, D], BF16, tag="lr")
                    nc.vector.tensor_tensor(out=lr[:], in0=z[:], in1=zs[:],
                                            op=mybir.AluOpType.max)
                    m = work.tile([P, D], BF16, tag="m")
                    nc.vector.tensor_tensor(out=m[:], in0=lr[:],
                                            in1=w_sb[att_rep][:],
                                            op=mybir.AluOpType.mult)
                    score = scp.tile([P, NH], F32, tag="score")
                    nc.vector.tensor_reduce(
                        out=score[:],
                        in_=m[:].rearrange("p (h c) -> p h c", h=NH),
                        axis=mybir.AxisListType.X, op=mybir.AluOpType.add,
                    )
                    rhs = rhsp.tile([P, D + NH], BF16, tag="rhs")
                    nc.scalar.activation(
                        out=rhs[:, D : D + NH], in_=score[:],
                        func=mybir.ActivationFunctionType.Exp,
                    )
                    p_b = rhs[:, D : D + NH].unsqueeze(2).broadcast_to([P, NH, C])
                    nc.vector.tensor_tensor(
                        out=rhs[:, :D].rearrange("p (h c) -> p h c", h=NH),
                        in0=G[:].rearrange("p (h c) -> p h c", h=NH),
                        in1=p_b, op=mybir.AluOpType.mult)
                    ind = work.tile([P, P], BF16, tag="ind")
                    nc.vector.tensor_tensor(
                        out=ind[:],
                        in0=dstloc_sb[:, k : k + 1].to_broadcast([P, P]),
                        in1=w_sb["iota128"][:],
                        op=mybir.AluOpType.is_equal,
                    )
                    nc.tensor.matmul(out=agg[:], lhsT=ind[:], rhs=rhs[:],
                                     start=(j == 0), stop=(j == S - 1))
                # ---- group epilogue
                den = ep.tile([P, NH], F32, tag="den")
                nc.vector.tensor_scalar(out=den[:], in0=agg[:, D : D + NH],
                                        scalar1=1e-30, scalar2=None,
                                        op0=mybir.AluOpType.max)
                rec = ep.tile([P, NH], F32, tag="rec")
                nc.vector.reciprocal(out=rec[:], in_=den[:])
                rec_b = rec[:].unsqueeze(2).broadcast_to([P, NH, C])
                outn = ep.tile([P, D], F32, tag="outn")
                nc.vector.tensor_tensor(
                    out=outn[:].rearrange("p (h c) -> p h c", h=NH),
                    in0=agg[:, :D].rearrange("p (h c) -> p h c", h=NH),
                    in1=rec_b, op=mybir.AluOpType.mult)
                nc.vector.tensor_tensor(out=outn[:], in0=outn[:],
                                        in1=w_sb[bias_rep][:],
                                        op=mybir.AluOpType.add)
                # elu: pos = max(outn,0); neg = min(outn,0); h = pos + (exp(neg)-1)
                neg = ep.tile([P, D], F32, tag="neg")
                nc.vector.tensor_scalar(out=neg[:], in0=outn[:], scalar1=0.0,
                                        scalar2=None, op0=mybir.AluOpType.min)
                en = ep.tile([P, D], F32, tag="en")
                nc.scalar.activation(out=en[:], in_=neg[:],
                                     func=mybir.ActivationFunctionType.Exp)
                nc.vector.tensor_scalar(out=en[:], in0=en[:], scalar1=-1.0,
                                        scalar2=None, op0=mybir.AluOpType.add)
                nc.vector.tensor_scalar(out=outn[:], in0=outn[:], scalar1=0.0,
                                        scalar2=None, op0=mybir.AluOpType.max)
                h = ep.tile([P, D], BF16, tag="h")
                nc.vector.tensor_tensor(out=h[:], in0=outn[:], in1=en[:],
                                        op=mybir.AluOpType.add)
                if pool_ctx is None:
                    # h1T resident: transpose h -> [feat, nodes]
                    tps = tpsum.tile([P, P], BF16, tag="tps")
                    nc.tensor.transpose(out=tps[:, :w], in_=h[:w, :],
                                        identity=ident_sb[:w, :w])
                    nc.scalar.activation(
                        out=h1T_sb[:, g * P : g * P + w], in_=tps[:, :w],
                        func=mybir.ActivationFunctionType.Copy,
                    )
                else:
                    (pool_psum,) = pool_ctx
                    pind = work.tile([P, NUM_GRAPHS], BF16, tag="pind")
                    nc.vector.tensor_tensor(
                        out=pind[:],
                        in0=batchloc_sb[:, g : g + 1].to_broadcast([P, NUM_GRAPHS]),
                        in1=w_sb["iota64"][:],
                        op=mybir.AluOpType.is_equal,
                    )
                    prhs = rhsp.tile([P, D + 1], BF16, tag="prhs")
                    nc.vector.tensor_copy(out=prhs[:, :D], in_=h[:])
                    nc.gpsimd.memset(prhs[:, D : D + 1], 1.0)
                    nc.tensor.matmul(out=pool_psum[:], lhsT=pind[:], rhs=prhs[:],
                                     start=(g == 0), stop=(g == ngroups - 1))

    with TileContext(nc) as tc:
        collective("AllGather", mybir.AluOpType.bypass, [xl1_own[:]], [xl1_full[:]])
        edge_layer(tc, xl1_full, xr1_own, "att1_rep", "bias1_rep", 8)

    with TileContext(nc) as tc:
        build_tables(tc, h1T_sb, "W2_l", "W2_r", "b2_l", "b2_r", xl2_own, xr2_own)

    with TileContext(nc) as tc:
        collective("AllGather", mybir.AluOpType.bypass, [xl2_own[:]], [xl2_full[:]])
        with tc.tile_pool(name="poolp", bufs=1, space="PSUM") as poolp, \
             tc.tile_pool(name="pstg", bufs=1) as pstg:
            pool_psum = poolp.tile([NUM_GRAPHS, 129], F32)
            edge_layer(tc, xl2_full, xr2_own, "att2_rep", "bias2_rep", 1,
                       pool_ctx=(pool_psum,))
            stg = pstg.tile([NUM_GRAPHS, 129], F32)
            nc.scalar.activation(out=stg[:], in_=pool_psum[:],
                                 func=mybir.ActivationFunctionType.Copy)
            nc.sync.dma_start(out=pool_stage[:], in_=stg[:])

    # ================= final head =================
    with TileContext(nc) as tc:
        collective("AllReduce", mybir.AluOpType.add, [pool_stage[:]], [pool_red[:]])
        with (
            tc.tile_pool(name="fin", bufs=1) as fin,
            tc.tile_pool(name="finp", bufs=1, space="PSUM") as finp,
        ):
            red = fin.tile([NUM_GRAPHS, 129], F32)
            nc.sync.dma_start(out=red[:], in_=pool_red[:])
            if debug:
                nc.sync.dma_start(out=dbg["dbg_red"][:], in_=red[:])
            cnt = fin.tile([NUM_GRAPHS, 1], F32)
            nc.vector.tensor_scalar(out=cnt[:], in0=red[:, 128:129], scalar1=1.0,
                                    scalar2=None, op0=mybir.AluOpType.max)
            rc = fin.tile([NUM_GRAPHS, 1], F32)
            nc.vector.reciprocal(out=rc[:], in_=cnt[:])
            pooled = fin.tile([NUM_GRAPHS, D], BF16)
            nc.vector.tensor_tensor(out=pooled[:], in0=red[:, :D],
                                    in1=rc[:].to_broadcast([NUM_GRAPHS, D]),
                                    op=mybir.AluOpType.mult)
            tp = finp.tile([P, NUM_GRAPHS], BF16)
            nc.tensor.transpose(out=tp[:], in_=pooled[:],
                                identity=ident_sb[:NUM_GRAPHS, :NUM_GRAPHS])
            pooledT = fin.tile([P, NUM_GRAPHS], BF16)
            nc.scalar.activation(out=pooledT[:], in_=tp[:],
                                 func=mybir.ActivationFunctionType.Copy)
            ops = finp.tile([NUM_GRAPHS, 10], F32)
            nc.tensor.matmul(out=ops[:], lhsT=pooledT[:], rhs=w_sb["W3"][:],
                             start=True, stop=False)
            nc.tensor.matmul(out=ops[:], lhsT=w_sb["ones1"][:, :NUM_GRAPHS],
                             rhs=w_sb["b3row"][:], start=False, stop=True)
            fout = fin.tile([NUM_GRAPHS, 10], F32)
            nc.scalar.activation(out=fout[:], in_=ops[:],
                                 func=mybir.ActivationFunctionType.Copy)
            nc.sync.dma_start(out=out_t[:], in_=fout[:])
        if debug:
            nc.sync.dma_start(out=dbg["dbg_xl1"][:], in_=xl1_own[:])
            nc.sync.dma_start(out=dbg["dbg_xr1"][:], in_=xr1_own[:])
            nc.sync.dma_start(out=dbg["dbg_xl1full"][:], in_=xl1_full[:])
            nc.sync.dma_start(out=dbg["dbg_h1T"][:], in_=h1T_sb[:])
            nc.sync.dma_start(out=dbg["dbg_xl2"][:], in_=xl2_own[:])
            nc.sync.dma_start(out=dbg["dbg_pool"][:], in_=pool_stage[:])

    _split_waits(nc)
    return nc


# ---------------------------------------------------------------- entry point
def _run(x, edge_index, batch, W1_l, b1_l, W1_r, b1_r, att1, bias1,
         W2_l, b2_l, W2_r, b2_r, att2, bias2, W3, b3, ncores=NCORES,
         debug=False, trace=False):
    x = np.asarray(x, np.float32)
    per_core, meta, perm = _prep(np.asarray(x), np.asarray(edge_index),
                                 np.asarray(batch), ncores)
    D = 128
    consts = dict(
        W1_l=np.asarray(W1_l, np.float32).astype(BF),
        W1_r=np.asarray(W1_r, np.float32).astype(BF),
        W2_l=np.asarray(W2_l, np.float32).astype(BF),
        W2_r=np.asarray(W2_r, np.float32).astype(BF),
        att1_rep=np.tile(np.asarray(att1, np.float32).reshape(1, D), (P, 1)).astype(BF),
        att2_rep=np.tile(np.asarray(att2, np.float32).reshape(1, D), (P, 1)).astype(BF),
        bias1_rep=np.tile(np.asarray(bias1, np.float32).reshape(1, D), (P, 1)).astype(BF),
        bias2_rep=np.tile(np.asarray(bias2, np.float32).reshape(1, D), (P, 1)).astype(BF),
        b1_l=np.asarray(b1_l, np.float32).reshape(1, D).astype(BF),
        b1_r=np.asarray(b1_r, np.float32).reshape(1, D).astype(BF),
        b2_l=np.asarray(b2_l, np.float32).reshape(1, D).astype(BF),
        b2_r=np.asarray(b2_r, np.float32).reshape(1, D).astype(BF),
        iota128=np.tile(np.arange(P, dtype=np.float32).reshape(1, P), (P, 1)).astype(BF),
        iota64=np.tile(np.arange(NUM_GRAPHS, dtype=np.float32).reshape(1, NUM_GRAPHS), (P, 1)).astype(BF),
        W3=np.asarray(W3, np.float32).astype(BF),
        b3row=np.asarray(b3, np.float32).reshape(1, 10).astype(BF),
        ones1=np.ones((1, P), np.float32).astype(BF),
    )
    nc = _build(meta, debug=debug)
    in_maps = []
    for c in range(ncores):
        m = dict(per_core[c])
        m.update(consts)
        in_maps.append(m)
    if trace:
        _install_profhook()
    res = run_bass_kernel_spmd(nc, in_maps, core_ids=list(range(ncores)),
                               trace=trace)
    return res.results[0]["out"].astype(np.float32), (res, per_core, meta, perm)


def kernel(**inputs):
    out, _res = _run(**inputs)
    return out



# revision 7
# speedup vs baseline: 1.6099x; 1.6099x over previous
"""2-layer GATv2 + global mean pool + linear head, on 8 Trainium2 NeuronCores.

Strategy (dst-sharded, degree-balanced, batched dma_gather):
  - Nodes are relabeled by a degree-balanced bin-packing into groups of <=128
    destination nodes with near-equal incident-edge counts.  Core c owns
    nodes [c*own, (c+1)*own); edges are assigned by destination owner.
  - Edges live in a static chunk grid (128 edge slots per chunk).  Each
    group's chunks are split into 4 source-subrange segments (int16
    dma_gather indices address <=32768 table rows) with per-(group,subrange)
    chunk budgets shared across cores, so one SPMD program fits all cores.
  - Blocks of 3 groups are processed at a time: batched dma_gather calls
    (<=4 chunks each, 4 SWDGE queues) fetch xl[src] (G) from the AllGather'd
    table and xr[dst] (V) from the core-local table; all elementwise work
    (z=G+V, leaky, att-mult, segment-score reduce, exp, p*G) runs as a few
    block-wide vector/ACT ops; a per-chunk PE matmul with the is_equal
    indicator scatters [p*G | p] into per-group PSUM accumulators (f32),
    giving numerators and softmax denominators at once.
  - Layer tables xl/xr = x@W + b are built on device; xl is AllGather'ed.
    Final pooling partial sums are AllReduce'd; every core computes the
    tiny output head.
"""

import sys

for _p in ("/opt/trn_rl_repo",):
    if _p not in sys.path:
        sys.path.insert(0, _p)

import numpy as np
import ml_dtypes

BF = ml_dtypes.bfloat16

import concourse.bass as bass
import concourse.mybir as mybir
from concourse.tile import TileContext
from concourse.bass_utils import run_bass_kernel_spmd
from concourse.masks import make_identity
from concourse import library_config
from concourse.library_overlay import lower_extended_insts

F32 = mybir.dt.float32
BF16 = mybir.dt.bfloat16
I16 = mybir.dt.int16
P = 128
D = 128
NCORES = 8
NUM_GRAPHS = 64
NEG_SLOPE = 0.2
SUBR = 25000          # int16 dma_gather window (4 x 25000 = 100000 rows)
NSUB = 4
GBLK = 2              # groups per processing block
CALL_CHUNKS = 4       # chunks per dma_gather call (512 rows)


# ---------------------------------------------------------------- prof hook
def _install_profhook():
    """Provide antenv.axon_hooks (absent in this image) so trace=True works."""
    import types

    if "antenv.axon_hooks" in sys.modules:
        return
    try:
        from trn_agent_boot.trn_boot import _ntff_profile_via_ctypes
    except Exception:
        return
    mod = types.ModuleType("antenv.axon_hooks")
    mod._hook = None
    mod.set_axon_ntff_profile_hook = lambda h: setattr(mod, "_hook", h)
    mod.get_axon_ntff_profile_hook = lambda: mod._hook
    sys.modules["antenv.axon_hooks"] = mod
    try:
        mod._hook = _ntff_profile_via_ctypes("/opt/axon/libaxon_pjrt.so")
    except Exception:
        mod._hook = None


# ---------------------------------------------------------------- wait split
def _split_waits(nc, max_waits=1):
    """walrus TPB_CTRL codegen rejects >1 sync-wait per instruction; move
    extras onto preceding NoOps on the same engine."""
    n_added = 0
    for fn in nc.m.functions:
        for blk in fn.blocks:
            new_insts = []
            for inst in blk.instructions:
                si = getattr(inst, "sync_info", None)
                waits = list(si.on_wait) if si is not None and si.on_wait else []
                if len(waits) > max_waits:
                    extra = waits[:-max_waits]
                    for i in range(0, len(extra), max_waits):
                        chunk = extra[i : i + max_waits]
                        nop = mybir.InstNoOp(
                            name=f"{inst.name}_wsplit{n_added}",
                            engine=inst.engine,
                            ins=[],
                            outs=[],
                            sync_info=mybir.SyncInfo(on_wait=chunk, on_update=[]),
                        )
                        n_added += 1
                        new_insts.append(nop)
                    si.on_wait = waits[-max_waits:]
                new_insts.append(inst)
            blk.instructions = new_insts
    return n_added


# ---------------------------------------------------------------- host prep
def _interleave16(vals):
    """[n] int16 -> [128, n/16] wrapped (i at [i%16, i//16]) and tiled x8."""
    n = vals.size
    a = np.zeros((16, n // 16), np.int16)
    a[np.arange(n) % 16, np.arange(n) // 16] = vals
    return np.tile(a, (8, 1))


def _prep(x, edge_index, batch, ncores):
    """Degree-balanced relabeling + blocked subrange chunk schedule."""
    N = x.shape[0]
    own = N // ncores
    gfull, rem = divmod(own, P)
    ngroups = gfull + (1 if rem else 0)

    src = np.concatenate([edge_index[0].astype(np.int64), np.arange(N)])
    dst = np.concatenate([edge_index[1].astype(np.int64), np.arange(N)])
    deg = np.bincount(dst, minlength=N)

    # degree-balanced bin packing (equal edge counts per group, all cores)
    caps = []
    for c in range(ncores):
        caps += [P] * gfull + ([rem] if rem else [])
    nbins = len(caps)
    import heapq

    heap = [(0, b) for b in range(nbins)]
    heapq.heapify(heap)
    bin_nodes = [[] for _ in range(nbins)]
    order = np.argsort(-deg, kind="stable")
    for node in order:
        while True:
            s, b = heapq.heappop(heap)
            if len(bin_nodes[b]) < caps[b]:
                break
        bin_nodes[b].append(node)
        if len(bin_nodes[b]) < caps[b]:
            heapq.heappush(heap, (s + int(deg[node]), b))

    perm = np.empty(N, np.int64)  # perm[new] = old
    for b in range(nbins):
        c, g = divmod(b, ngroups)
        base = c * own + g * P
        nodes = bin_nodes[b]
        perm[base : base + len(nodes)] = nodes
    inv = np.empty(N, np.int64)
    inv[perm] = np.arange(N)

    new_src = inv[src]
    new_dst = inv[dst]
    core_of = new_dst // own

    # per (core, group, subrange) edge lists and counts
    sub_of = new_src // SUBR
    n_cgr = np.zeros((ncores, ngroups, NSUB), np.int64)
    for c in range(ncores):
        m = core_of == c
        gg = (new_dst[m] % own) // P
        rr = sub_of[m]
        np.add.at(n_cgr[c], (gg, rr), 1)
    B = np.ceil(n_cgr.max(axis=0) / P).astype(np.int64)  # [ngroups, NSUB]
    S_list = B.sum(axis=1)  # chunks per group

    # block schedule (shared across cores)
    blocks = []
    tot_chunks = 0
    g0 = 0
    while g0 < ngroups:
        groups = list(range(g0, min(g0 + GBLK, ngroups)))
        chunk_group = []       # tile-chunk -> group
        chunk_sub = []         # tile-chunk -> subrange
        seg_start = {}         # (g, r) -> first tile-chunk (block-local)
        for r in range(NSUB):
            for g in groups:
                seg_start[(g, r)] = len(chunk_group)
                chunk_group += [g] * int(B[g, r])
                chunk_sub += [r] * int(B[g, r])
        CB = len(chunk_group)
        # gather calls: per r-region, sub-calls of <= CALL_CHUNKS chunks
        gcalls = []
        pos = 0
        for r in range(NSUB):
            nreg = int(B[groups, r].sum() if hasattr(B, "sum") else 0)
            nreg = int(sum(int(B[g, r]) for g in groups))
            q = 0
            while q < nreg:
                n = min(CALL_CHUNKS, nreg - q)
                gcalls.append((r, pos + q, n))
                q += n
            pos += nreg
        vcalls = []
        q = 0
        while q < CB:
            n = min(CALL_CHUNKS, CB - q)
            vcalls.append((q, n))
            q += n
        gfirst = {g: min(i for i, gg in enumerate(chunk_group) if gg == g)
                  for g in groups}
        glast = {g: max(i for i, gg in enumerate(chunk_group) if gg == g)
                 for g in groups}
        blocks.append(dict(groups=groups, CB=CB, chunk_group=chunk_group,
                           chunk_sub=chunk_sub, seg_start=seg_start,
                           gcalls=gcalls, vcalls=vcalls,
                           gfirst=gfirst, glast=glast,
                           chunk0=tot_chunks))
        tot_chunks += CB
        g0 += GBLK

    CBmax = max(b["CB"] for b in blocks)
    idxw = tot_chunks * 8  # int16 idx columns (128 rows per chunk / 16)

    # per-core static arrays
    per_core = []
    for c in range(ncores):
        m = core_of == c
        es = new_src[m]
        ed_own = new_dst[m] % own
        gg = ed_own // P
        rr = es // SUBR
        # order edges by (g, r) stable; k = running index within (g,r)
        o = np.lexsort((np.arange(es.size), rr, gg))
        es_s, ed_s, gg_s, rr_s = es[o], ed_own[o], gg[o], rr[o]
        key = gg_s * NSUB + rr_s
        firstk = np.r_[True, key[1:] != key[:-1]]
        startk = np.maximum.accumulate(np.where(firstk, np.arange(key.size), 0))
        k = np.arange(key.size) - startk

        srcidx_flat = np.zeros(tot_chunks * P, np.int16)
        dstidx_flat = np.zeros(tot_chunks * P, np.int16)
        dstloc = np.full((P, tot_chunks), 255.0, np.float32)

        # tile chunk for each edge
        blk_of_g = gg_s // GBLK
        seg0 = np.zeros(es_s.size, np.int64)
        chunk0 = np.zeros(es_s.size, np.int64)
        for bi, b in enumerate(blocks):
            mm = blk_of_g == bi
            if not mm.any():
                continue
            ss = np.array([b["seg_start"][(int(g), int(r))]
                           for g, r in zip(gg_s[mm], rr_s[mm])])
            seg0[mm] = ss + b["chunk0"]
        tchunk = seg0 + k // P
        p_e = k % P
        flat = tchunk * P + p_e
        srcidx_flat[flat] = (es_s - rr_s * SUBR).astype(np.int16)
        dstidx_flat[flat] = ed_s.astype(np.int16)
        dstloc.ravel()[p_e * tot_chunks + tchunk] = (ed_s - gg_s * P).astype(
            np.float32)

        # wrapped idx arrays per call (concatenated windows, 8 cols per chunk)
        srcidx16 = np.zeros((P, idxw), np.int16)
        dstidx16 = np.zeros((P, idxw), np.int16)
        for b in blocks:
            c0b = b["chunk0"]
            for (r, q, n) in b["gcalls"]:
                w0 = (c0b + q) * 8
                vals = srcidx_flat[(c0b + q) * P : (c0b + q + n) * P]
                srcidx16[:, w0 : w0 + n * 8] = _interleave16(vals)
            for (q, n) in b["vcalls"]:
                w0 = (c0b + q) * 8
                vals = dstidx_flat[(c0b + q) * P : (c0b + q + n) * P]
                dstidx16[:, w0 : w0 + n * 8] = _interleave16(vals)

        xT_own = np.ascontiguousarray(x[perm[c * own : (c + 1) * own]].T).astype(BF)

        bl = np.full((P, ngroups), 255.0, np.float32)
        for g in range(ngroups):
            size = P if (g < gfull or rem == 0) else rem
            ids = perm[c * own + g * P : c * own + g * P + size]
            bl[:size, g] = batch[ids]

        per_core.append(dict(
            srcidx16=srcidx16, dstidx16=dstidx16,
            dstloc=dstloc.astype(BF), xT_own=xT_own,
            batchloc=bl.astype(BF),
        ))

    meta = dict(N=N, own=own, ngroups=ngroups, gfull=gfull, rem=rem,
                ncores=ncores, blocks=blocks, tot_chunks=tot_chunks,
                CBmax=CBmax, idxw=idxw,
                S_list=[int(v) for v in S_list])
    return per_core, meta, perm


def _gsize(meta, g):
    return P if (g < meta["gfull"] or meta["rem"] == 0) else meta["rem"]


# ---------------------------------------------------------------- kernel build
def _build(meta, debug=False):
    own = meta["own"]
    ngroups = meta["ngroups"]
    ncores = meta["ncores"]
    blocks = meta["blocks"]
    tot_chunks = meta["tot_chunks"]
    CBmax = meta["CBmax"]
    idxw = meta["idxw"]

    nc = bass.Bass(target_bir_lowering=False, debug=True, num_swdge_queues=4)

    # ---- external inputs (per core)
    xT_in = nc.declare_dram_parameter("xT_own", [P, own], BF16, isOutput=False)
    srcidx_in = nc.declare_dram_parameter("srcidx16", [P, idxw], I16, isOutput=False)
    dstidx_in = nc.declare_dram_parameter("dstidx16", [P, idxw], I16, isOutput=False)
    dstloc_in = nc.declare_dram_parameter("dstloc", [P, tot_chunks], BF16,
                                          isOutput=False)
    batchloc_in = nc.declare_dram_parameter("batchloc", [P, ngroups], BF16,
                                            isOutput=False)
    wnames = [
        ("W1_l", [P, D]), ("W1_r", [P, D]), ("W2_l", [P, D]), ("W2_r", [P, D]),
        ("att1_rep", [P, D]), ("att2_rep", [P, D]),
        ("bias1_rep", [P, D]), ("bias2_rep", [P, D]),
        ("b1_l", [1, D]), ("b1_r", [1, D]), ("b2_l", [1, D]), ("b2_r", [1, D]),
        ("iota128", [P, P]), ("iota64", [P, NUM_GRAPHS]),
        ("W3", [P, 10]), ("b3row", [1, 10]), ("ones1", [1, P]),
        ("ones_col", [P, 1]),
    ]
    w_in = {n: nc.declare_dram_parameter(n, sh, BF16, isOutput=False)
            for n, sh in wnames}
    out_t = nc.declare_dram_parameter("out", [NUM_GRAPHS, 10], F32, isOutput=True)
    dbg = {}
    if debug:
        dbg["dbg_h1T"] = nc.declare_dram_parameter("dbg_h1T", [P, own], BF16,
                                                   isOutput=True)

    # ---- internal DRAM
    xl1_own = nc.dram_tensor("xl1_own", [own, D], BF16)
    xr1_own = nc.dram_tensor("xr1_own", [own, D], BF16)
    xl1_full = nc.dram_tensor("xl1_full", [own * ncores, D], BF16,
                              addr_space="Shared")
    xl2_own = nc.dram_tensor("xl2_own", [own, D], BF16)
    xr2_own = nc.dram_tensor("xr2_own", [own, D], BF16)
    xl2_full = nc.dram_tensor("xl2_full", [own * ncores, D], BF16,
                              addr_space="Shared")
    pool_stage = nc.dram_tensor("pool_stage", [NUM_GRAPHS, D + 1], F32)
    pool_red = nc.dram_tensor("pool_red", [NUM_GRAPHS, D + 1], F32,
                              addr_space="Shared")

    # ---- persistent SBUF
    xT_sb = nc.alloc_sbuf_tensor("xT_sb", [P, own], BF16)
    h1T_sb = nc.alloc_sbuf_tensor("h1T_sb", [P, own], BF16)
    dstloc_sb = nc.alloc_sbuf_tensor("dstloc_sb", [P, tot_chunks], BF16)
    batchloc_sb = nc.alloc_sbuf_tensor("batchloc_sb", [P, ngroups], BF16)
    ident_sb = nc.alloc_sbuf_tensor("ident_sb", [P, P], BF16)
    w_sb = {n: nc.alloc_sbuf_tensor(n + "_sb", sh, BF16) for n, sh in wnames}

    nreg = nc.gpsimd.alloc_register("nidx")

    def collective(kind, op, ins, outs):
        nc.gpsimd.collective_compute(
            kind, op, replica_groups=[list(range(ncores))], ins=ins, outs=outs
        )

    # ================= TC-load =================
    with TileContext(nc) as tc:
        with tc.tile_pool(name="idp", bufs=1) as idp:
            idt = idp.tile([P, P], F32)
            make_identity(nc, idt[:])
            nc.vector.tensor_copy(out=ident_sb[:], in_=idt[:])
        nc.gpsimd.load_library(library_config.mlp)
        nc.sync.dma_start(out=xT_sb[:], in_=xT_in[:])
        nc.sync.dma_start(out=dstloc_sb[:], in_=dstloc_in[:])
        nc.sync.dma_start(out=batchloc_sb[:], in_=batchloc_in[:])
        for n, _sh in wnames:
            nc.sync.dma_start(out=w_sb[n][:], in_=w_in[n][:])

    # ================= table build =================
    def build_tables(tc, srcT_sb, Wl, Wr, bl, br, out_l, out_r):
        with (
            tc.tile_pool(name="tp", bufs=3) as tp,
            tc.tile_pool(name="tpp", bufs=3, space="PSUM") as tpp,
        ):
            for g in range(ngroups):
                w = _gsize(meta, g)
                for W, brow, dest in ((Wl, bl, out_l), (Wr, br, out_r)):
                    ps = tpp.tile([P, D], F32, tag="ps")
                    nc.tensor.matmul(out=ps[:w, :],
                                     lhsT=srcT_sb[:, g * P : g * P + w],
                                     rhs=w_sb[W][:], start=True, stop=False)
                    nc.tensor.matmul(out=ps[:w, :], lhsT=w_sb["ones1"][:, :w],
                                     rhs=w_sb[brow][:], start=False, stop=True)
                    ot = tp.tile([P, D], BF16, tag="ot")
                    nc.scalar.activation(out=ot[:w, :], in_=ps[:w, :],
                                         func=mybir.ActivationFunctionType.Copy)
                    nc.sync.dma_start(out=dest[g * P : g * P + w, :],
                                      in_=ot[:w, :])

    # ================= edge layer =================
    def edge_layer(tc, xl_full_t, xr_own_t, att_rep, bias_rep, NH,
                   pool_ctx=None):
        C = D // NH
        Q = D + NH
        qn = [0]

        def next_q():
            qn[0] = (qn[0] + 1) % 4
            return qn[0]

        with (
            tc.tile_pool(name="gp", bufs=2) as gp,
            tc.tile_pool(name="vp", bufs=2) as vp,
            tc.tile_pool(name="rp", bufs=2) as rp,
            tc.tile_pool(name="ip", bufs=2) as ip,
            tc.tile_pool(name="indp", bufs=2) as indp,
            tc.tile_pool(name="sp", bufs=2) as sp,
            tc.tile_pool(name="ep", bufs=2) as ep,
            tc.tile_pool(name="aggp", bufs=2, space="PSUM") as aggp,
            tc.tile_pool(name="tpsum", bufs=2, space="PSUM") as tpsum,
        ):
            for b in blocks:
                CB = b["CB"]
                c0b = b["chunk0"]
                G = gp.tile([P, CBmax * D], BF16, tag="G")
                V = vp.tile([P, CBmax * D], BF16, tag="V")
                gidx = ip.tile([P, CBmax * 8], I16, tag="gidx")
                vidx = ip.tile([P, CBmax * 8], I16, tag="vidx")
                nc.sync.dma_start(out=gidx[:, : CB * 8],
                                  in_=srcidx_in[:, c0b * 8 : (c0b + CB) * 8])
                nc.sync.dma_start(out=vidx[:, : CB * 8],
                                  in_=dstidx_in[:, c0b * 8 : (c0b + CB) * 8])
                # gathers
                for (r, q, n) in b["gcalls"]:
                    nc.gpsimd.reg_mov(nreg, n * P)
                    nc.gpsimd.dma_gather(
                        G[:, q * D : (q + n) * D].rearrange(
                            "p (j c) -> p j c", c=D),
                        xl_full_t[r * SUBR : min((r + 1) * SUBR, own * ncores), :],
                        gidx[:, q * 8 : (q + n) * 8],
                        n * P, nreg, D, single_packet=False, queue_num=next_q())
                for (q, n) in b["vcalls"]:
                    nc.gpsimd.reg_mov(nreg, n * P)
                    nc.gpsimd.dma_gather(
                        V[:, q * D : (q + n) * D].rearrange(
                            "p (j c) -> p j c", c=D),
                        xr_own_t[:],
                        vidx[:, q * 8 : (q + n) * 8],
                        n * P, nreg, D, single_packet=False, queue_num=next_q())
                Gs = G[:, : CB * D]
                Vs = V[:, : CB * D]
                G3 = Gs.rearrange("p (m c) -> p m c", c=D)
                V3 = Vs.rearrange("p (m c) -> p m c", c=D)
                # z = G + V  (into V)
                nc.vector.tensor_tensor(out=V3, in0=G3, in1=V3,
                                        op=mybir.AluOpType.add)
                # leaky in place
                nc.vector.scalar_tensor_tensor(
                    out=Vs, in0=Vs, scalar=NEG_SLOPE, in1=Vs,
                    op0=mybir.AluOpType.mult, op1=mybir.AluOpType.max)
                # m = lr * att
                att_b = w_sb[att_rep][:].unsqueeze(1).broadcast_to([P, CB, D])
                nc.vector.tensor_tensor(out=V3, in0=V3, in1=att_b,
                                        op=mybir.AluOpType.mult)
                # score
                SC = sp.tile([P, CBmax * NH], F32, tag="SC")
                SCs = SC[:, : CB * NH]
                nc.vector.tensor_reduce(
                    out=SCs, in_=Vs.rearrange("p (mh c) -> p mh c", c=C),
                    axis=mybir.AxisListType.X, op=mybir.AluOpType.add)
                # p = exp(score): into PT (for p*G) and into RHS cols D..D+NH
                PT = sp.tile([P, CBmax * NH], BF16, tag="PT")
                PTs = PT[:, : CB * NH]
                nc.scalar.activation(out=PTs, in_=SCs,
                                     func=mybir.ActivationFunctionType.Exp)
                RHS = rp.tile([P, CBmax * Q], BF16, tag="RHS")
                RQ = RHS[:, : CB * Q].rearrange("p (t q) -> p t q", q=Q)
                nc.scalar.activation(
                    out=RQ[:, :, D : D + NH],
                    in_=SCs.rearrange("p (m h) -> p m h", h=NH),
                    func=mybir.ActivationFunctionType.Exp)
                # rhs = G * p
                p_b = PTs.rearrange("p (m h) -> p m h", h=NH).unsqueeze(
                    3).broadcast_to([P, CB, NH, C])
                nc.vector.tensor_tensor(
                    out=RQ[:, :, :D].rearrange("p t (h c) -> p t h c", h=NH),
                    in0=Gs.rearrange("p (m h c) -> p m h c", h=NH, c=C),
                    in1=p_b, op=mybir.AluOpType.mult)
                # indicator
                IND = indp.tile([P, CBmax * D], BF16, tag="IND")
                nc.vector.tensor_tensor(
                    out=IND[:, : CB * D].rearrange("p (t l) -> p t l", l=P),
                    in0=dstloc_sb[:, c0b : c0b + CB].unsqueeze(2).broadcast_to(
                        [P, CB, P]),
                    in1=w_sb["iota128"][:].unsqueeze(1).broadcast_to([P, CB, P]),
                    op=mybir.AluOpType.is_equal)
                # scatter matmuls + per-group epilogue
                aggs = {}
                for g in b["groups"]:
                    agt = aggp.tile([P, Q], F32, tag=f"agg{g % GBLK}",
                                    name=f"agg{g % GBLK}")
                    aggs[g] = agt
                for t in range(CB):
                    g = b["chunk_group"][t]
                    nc.tensor.matmul(
                        out=aggs[g][:],
                        lhsT=IND[:, t * D : (t + 1) * D],
                        rhs=RHS[:, t * Q : (t + 1) * Q],
                        start=(t == b["gfirst"][g]),
                        stop=(t == b["glast"][g]))
                for g in b["groups"]:
                    agg = aggs[g]
                    w = _gsize(meta, g)
                    DEN = ep.tile([P, NH], F32, tag="DEN")
                    nc.vector.tensor_scalar(out=DEN[:], in0=agg[:, D : D + NH],
                                            scalar1=1e-30, scalar2=None,
                                            op0=mybir.AluOpType.max)
                    REC = ep.tile([P, NH], F32, tag="REC")
                    nc.vector.reciprocal(out=REC[:], in_=DEN[:])
                    OUT = ep.tile([P, D], F32, tag="OUT")
                    rec_b = REC[:].unsqueeze(2).broadcast_to([P, NH, C])
                    nc.vector.tensor_tensor(
                        out=OUT[:].rearrange("p (h c) -> p h c", h=NH),
                        in0=agg[:, :D].rearrange("p (h c) -> p h c", h=NH),
                        in1=rec_b, op=mybir.AluOpType.mult)
                    nc.vector.tensor_tensor(out=OUT[:], in0=OUT[:],
                                            in1=w_sb[bias_rep][:],
                                            op=mybir.AluOpType.add)
                    NEG = ep.tile([P, D], F32, tag="NEG")
                    nc.vector.tensor_scalar(out=NEG[:], in0=OUT[:], scalar1=0.0,
                                            scalar2=None,
                                            op0=mybir.AluOpType.min)
                    EN = ep.tile([P, D], F32, tag="EN")
                    nc.scalar.activation(out=EN[:], in_=NEG[:],
                                         func=mybir.ActivationFunctionType.Exp)
                    nc.vector.tensor_scalar(out=OUT[:], in0=OUT[:], scalar1=0.0,
                                            scalar2=None,
                                            op0=mybir.AluOpType.max)
                    H = ep.tile([P, D], BF16, tag="H")
                    nc.vector.scalar_tensor_tensor(
                        out=H[:], in0=EN[:], scalar=-1.0, in1=OUT[:],
                        op0=mybir.AluOpType.add, op1=mybir.AluOpType.add)
                    if pool_ctx is None:
                        tps = tpsum.tile([P, P], BF16, tag="tps")
                        nc.tensor.transpose(out=tps[:, :w], in_=H[:w, :],
                                            identity=ident_sb[:w, :w])
                        nc.scalar.activation(
                            out=h1T_sb[:, g * P : g * P + w], in_=tps[:, :w],
                            func=mybir.ActivationFunctionType.Copy)
                    else:
                        (pool_psum,) = pool_ctx
                        pind = ep.tile([P, NUM_GRAPHS], BF16, tag="pind")
                        nc.vector.tensor_tensor(
                            out=pind[:],
                            in0=batchloc_sb[:, g : g + 1].to_broadcast(
                                [P, NUM_GRAPHS]),
                            in1=w_sb["iota64"][:],
                            op=mybir.AluOpType.is_equal)
                        prhs = ep.tile([P, D + 1], BF16, tag="prhs")
                        nc.vector.tensor_copy(out=prhs[:, :D], in_=H[:])
                        nc.vector.tensor_copy(out=prhs[:, D : D + 1],
                                              in_=w_sb["ones_col"][:])
                        nc.tensor.matmul(out=pool_psum[:], lhsT=pind[:],
                                         rhs=prhs[:],
                                         start=(g == 0),
                                         stop=(g == ngroups - 1))

    # ================= layer 1 =================
    with TileContext(nc) as tc:
        build_tables(tc, xT_sb, "W1_l", "W1_r", "b1_l", "b1_r",
                     xl1_own, xr1_own)

    with TileContext(nc) as tc:
        collective("AllGather", mybir.AluOpType.bypass, [xl1_own[:]],
                   [xl1_full[:]])
        edge_layer(tc, xl1_full, xr1_own, "att1_rep", "bias1_rep", 8)

    # ================= layer 2 =================
    with TileContext(nc) as tc:
        build_tables(tc, h1T_sb, "W2_l", "W2_r", "b2_l", "b2_r",
                     xl2_own, xr2_own)

    with TileContext(nc) as tc:
        collective("AllGather", mybir.AluOpType.bypass, [xl2_own[:]],
                   [xl2_full[:]])
        with tc.tile_pool(name="poolp", bufs=1, space="PSUM") as poolp, \
             tc.tile_pool(name="pstg", bufs=1) as pstg:
            pool_psum = poolp.tile([NUM_GRAPHS, D + 1], F32)
            edge_layer(tc, xl2_full, xr2_own, "att2_rep", "bias2_rep", 1,
                       pool_ctx=(pool_psum,))
            stg = pstg.tile([NUM_GRAPHS, D + 1], F32)
            nc.scalar.activation(out=stg[:], in_=pool_psum[:],
                                 func=mybir.ActivationFunctionType.Copy)
            nc.sync.dma_start(out=pool_stage[:], in_=stg[:])

    # ================= final head =================
    with TileContext(nc) as tc:
        collective("AllReduce", mybir.AluOpType.add, [pool_stage[:]],
                   [pool_red[:]])
        with (
            tc.tile_pool(name="fin", bufs=1) as fin,
            tc.tile_pool(name="finp", bufs=1, space="PSUM") as finp,
        ):
            red = fin.tile([NUM_GRAPHS, D + 1], F32)
            nc.sync.dma_start(out=red[:], in_=pool_red[:])
            cnt = fin.tile([NUM_GRAPHS, 1], F32)
            nc.vector.tensor_scalar(out=cnt[:], in0=red[:, D : D + 1],
                                    scalar1=1.0, scalar2=None,
                                    op0=mybir.AluOpType.max)
            rc = fin.tile([NUM_GRAPHS, 1], F32)
            nc.vector.reciprocal(out=rc[:], in_=cnt[:])
            pooled = fin.tile([NUM_GRAPHS, D], BF16)
            nc.vector.tensor_tensor(out=pooled[:], in0=red[:, :D],
                                    in1=rc[:].to_broadcast([NUM_GRAPHS, D]),
                                    op=mybir.AluOpType.mult)
            tp = finp.tile([P, NUM_GRAPHS], BF16)
            nc.tensor.transpose(out=tp[:], in_=pooled[:],
                                identity=ident_sb[:NUM_GRAPHS, :NUM_GRAPHS])
            pooledT = fin.tile([P, NUM_GRAPHS], BF16)
            nc.scalar.activation(out=pooledT[:], in_=tp[:],
                                 func=mybir.ActivationFunctionType.Copy)
            ops = finp.tile([NUM_GRAPHS, 10], F32)
            nc.tensor.matmul(out=ops[:], lhsT=pooledT[:], rhs=w_sb["W3"][:],
                             start=True, stop=False)
            nc.tensor.matmul(out=ops[:], lhsT=w_sb["ones1"][:, :NUM_GRAPHS],
                             rhs=w_sb["b3row"][:], start=False, stop=True)
            fout = fin.tile([NUM_GRAPHS, 10], F32)
            nc.scalar.activation(out=fout[:], in_=ops[:],
                                 func=mybir.ActivationFunctionType.Copy)
            nc.sync.dma_start(out=out_t[:], in_=fout[:])
        if debug:
            nc.sync.dma_start(out=dbg["dbg_h1T"][:], in_=h1T_sb[:])

    lower_extended_insts(nc)
    _split_waits(nc)
    return nc


# ---------------------------------------------------------------- entry point
def _run(x, edge_index, batch, W1_l, b1_l, W1_r, b1_r, att1, bias1,
         W2_l, b2_l, W2_r, b2_r, att2, bias2, W3, b3, ncores=NCORES,
         debug=False, trace=False):
    x = np.asarray(x, np.float32)
    per_core, meta, perm = _prep(np.asarray(x), np.asarray(edge_index),
                                 np.asarray(batch), ncores)
    consts = dict(
        W1_l=np.asarray(W1_l, np.float32).astype(BF),
        W1_r=np.asarray(W1_r, np.float32).astype(BF),
        W2_l=np.asarray(W2_l, np.float32).astype(BF),
        W2_r=np.asarray(W2_r, np.float32).astype(BF),
        att1_rep=np.tile(np.asarray(att1, np.float32).reshape(1, D),
                         (P, 1)).astype(BF),
        att2_rep=np.tile(np.asarray(att2, np.float32).reshape(1, D),
                         (P, 1)).astype(BF),
        bias1_rep=np.tile(np.asarray(bias1, np.float32).reshape(1, D),
                          (P, 1)).astype(BF),
        bias2_rep=np.tile(np.asarray(bias2, np.float32).reshape(1, D),
                          (P, 1)).astype(BF),
        b1_l=np.asarray(b1_l, np.float32).reshape(1, D).astype(BF),
        b1_r=np.asarray(b1_r, np.float32).reshape(1, D).astype(BF),
        b2_l=np.asarray(b2_l, np.float32).reshape(1, D).astype(BF),
        b2_r=np.asarray(b2_r, np.float32).reshape(1, D).astype(BF),
        iota128=np.tile(np.arange(P, dtype=np.float32).reshape(1, P),
                        (P, 1)).astype(BF),
        iota64=np.tile(np.arange(NUM_GRAPHS, dtype=np.float32).reshape(
            1, NUM_GRAPHS), (P, 1)).astype(BF),
        W3=np.asarray(W3, np.float32).astype(BF),
        b3row=np.asarray(b3, np.float32).reshape(1, 10).astype(BF),
        ones1=np.ones((1, P), np.float32).astype(BF),
        ones_col=np.ones((P, 1), np.float32).astype(BF),
    )
    nc = _build(meta, debug=debug)
    in_maps = []
    for c in range(ncores):
        m = dict(per_core[c])
        m.update(consts)
        in_maps.append(m)
    if trace:
        _install_profhook()
    res = run_bass_kernel_spmd(nc, in_maps, core_ids=list(range(ncores)),
                               trace=trace)
    return res.results[0]["out"].astype(np.float32), (res, per_core, meta, perm)


def kernel(**inputs):
    out, _res = _run(**inputs)
    return out


# revision 8
# speedup vs baseline: 1.7273x; 1.0730x over previous
"""2-layer GATv2 + global mean pool + linear head, on 8 Trainium2 NeuronCores.

Strategy (dst-sharded, degree-balanced, batched dma_gather):
  - Nodes are relabeled by a degree-balanced bin-packing into groups of <=128
    destination nodes with near-equal incident-edge counts.  Core c owns
    nodes [c*own, (c+1)*own); edges are assigned by destination owner.
  - Edges live in a static chunk grid (128 edge slots per chunk).  Each
    group's chunks are split into 4 source-subrange segments (int16
    dma_gather indices address <=32768 table rows) with per-(group,subrange)
    chunk budgets shared across cores, so one SPMD program fits all cores.
  - Blocks of 3 groups are processed at a time: batched dma_gather calls
    (<=4 chunks each, 4 SWDGE queues) fetch xl[src] (G) from the AllGather'd
    table and xr[dst] (V) from the core-local table; all elementwise work
    (z=G+V, leaky, att-mult, segment-score reduce, exp, p*G) runs as a few
    block-wide vector/ACT ops; a per-chunk PE matmul with the is_equal
    indicator scatters [p*G | p] into per-group PSUM accumulators (f32),
    giving numerators and softmax denominators at once.
  - Layer tables xl/xr = x@W + b are built on device; xl is AllGather'ed.
    Final pooling partial sums are AllReduce'd; every core computes the
    tiny output head.
"""

import sys

for _p in ("/opt/trn_rl_repo",):
    if _p not in sys.path:
        sys.path.insert(0, _p)

import numpy as np
import ml_dtypes

BF = ml_dtypes.bfloat16

import concourse.bass as bass
import concourse.mybir as mybir
from concourse.tile import TileContext
from concourse.bass_utils import run_bass_kernel_spmd
from concourse.masks import make_identity
from concourse import library_config
from concourse.library_overlay import lower_extended_insts

F32 = mybir.dt.float32
BF16 = mybir.dt.bfloat16
I16 = mybir.dt.int16
P = 128
D = 128
NCORES = 8
NUM_GRAPHS = 64
NEG_SLOPE = 0.2
SUBR = 25000          # int16 dma_gather window (4 x 25000 = 100000 rows)
NSUB = 4
GBLK = 2              # groups per processing block
CALL_CHUNKS = 4       # chunks per dma_gather call (512 rows)


# ---------------------------------------------------------------- prof hook
def _install_profhook():
    """Provide antenv.axon_hooks (absent in this image) so trace=True works."""
    import types

    if "antenv.axon_hooks" in sys.modules:
        return
    try:
        from trn_agent_boot.trn_boot import _ntff_profile_via_ctypes
    except Exception:
        return
    mod = types.ModuleType("antenv.axon_hooks")
    mod._hook = None
    mod.set_axon_ntff_profile_hook = lambda h: setattr(mod, "_hook", h)
    mod.get_axon_ntff_profile_hook = lambda: mod._hook
    sys.modules["antenv.axon_hooks"] = mod
    try:
        mod._hook = _ntff_profile_via_ctypes("/opt/axon/libaxon_pjrt.so")
    except Exception:
        mod._hook = None


# ---------------------------------------------------------------- wait split
def _split_waits(nc, max_waits=1):
    """walrus TPB_CTRL codegen rejects >1 sync-wait per instruction; move
    extras onto preceding NoOps on the same engine."""
    n_added = 0
    for fn in nc.m.functions:
        for blk in fn.blocks:
            new_insts = []
            for inst in blk.instructions:
                si = getattr(inst, "sync_info", None)
                waits = list(si.on_wait) if si is not None and si.on_wait else []
                if len(waits) > max_waits:
                    extra = waits[:-max_waits]
                    for i in range(0, len(extra), max_waits):
                        chunk = extra[i : i + max_waits]
                        nop = mybir.InstNoOp(
                            name=f"{inst.name}_wsplit{n_added}",
                            engine=inst.engine,
                            ins=[],
                            outs=[],
                            sync_info=mybir.SyncInfo(on_wait=chunk, on_update=[]),
                        )
                        n_added += 1
                        new_insts.append(nop)
                    si.on_wait = waits[-max_waits:]
                new_insts.append(inst)
            blk.instructions = new_insts
    return n_added


# ---------------------------------------------------------------- host prep
def _interleave16(vals):
    """[n] int16 -> [128, n/16] wrapped (i at [i%16, i//16]) and tiled x8."""
    n = vals.size
    a = np.zeros((16, n // 16), np.int16)
    a[np.arange(n) % 16, np.arange(n) // 16] = vals
    return np.tile(a, (8, 1))


def _prep(x, edge_index, batch, ncores):
    """Degree-balanced relabeling + blocked subrange chunk schedule."""
    N = x.shape[0]
    own = N // ncores
    gfull, rem = divmod(own, P)
    ngroups = gfull + (1 if rem else 0)

    src = np.concatenate([edge_index[0].astype(np.int64), np.arange(N)])
    dst = np.concatenate([edge_index[1].astype(np.int64), np.arange(N)])
    deg = np.bincount(dst, minlength=N)

    # degree-balanced bin packing (equal edge counts per group, all cores)
    caps = []
    for c in range(ncores):
        caps += [P] * gfull + ([rem] if rem else [])
    nbins = len(caps)
    import heapq

    heap = [(0, b) for b in range(nbins)]
    heapq.heapify(heap)
    bin_nodes = [[] for _ in range(nbins)]
    order = np.argsort(-deg, kind="stable")
    for node in order:
        while True:
            s, b = heapq.heappop(heap)
            if len(bin_nodes[b]) < caps[b]:
                break
        bin_nodes[b].append(node)
        if len(bin_nodes[b]) < caps[b]:
            heapq.heappush(heap, (s + int(deg[node]), b))

    perm = np.empty(N, np.int64)  # perm[new] = old
    for b in range(nbins):
        c, g = divmod(b, ngroups)
        base = c * own + g * P
        nodes = bin_nodes[b]
        perm[base : base + len(nodes)] = nodes
    inv = np.empty(N, np.int64)
    inv[perm] = np.arange(N)

    new_src = inv[src]
    new_dst = inv[dst]
    core_of = new_dst // own

    # per (core, group, subrange) edge lists and counts
    sub_of = new_src // SUBR
    n_cgr = np.zeros((ncores, ngroups, NSUB), np.int64)
    for c in range(ncores):
        m = core_of == c
        gg = (new_dst[m] % own) // P
        rr = sub_of[m]
        np.add.at(n_cgr[c], (gg, rr), 1)
    B = np.ceil(n_cgr.max(axis=0) / P).astype(np.int64)  # [ngroups, NSUB]
    S_list = B.sum(axis=1)  # chunks per group

    # block schedule (shared across cores)
    blocks = []
    tot_chunks = 0
    g0 = 0
    while g0 < ngroups:
        groups = list(range(g0, min(g0 + GBLK, ngroups)))
        chunk_group = []       # tile-chunk -> group
        chunk_sub = []         # tile-chunk -> subrange
        seg_start = {}         # (g, r) -> first tile-chunk (block-local)
        for r in range(NSUB):
            for g in groups:
                seg_start[(g, r)] = len(chunk_group)
                chunk_group += [g] * int(B[g, r])
                chunk_sub += [r] * int(B[g, r])
        CB = len(chunk_group)
        # gather calls: per r-region, sub-calls of <= CALL_CHUNKS chunks
        gcalls = []
        pos = 0
        for r in range(NSUB):
            nreg = int(B[groups, r].sum() if hasattr(B, "sum") else 0)
            nreg = int(sum(int(B[g, r]) for g in groups))
            q = 0
            while q < nreg:
                n = min(CALL_CHUNKS, nreg - q)
                gcalls.append((r, pos + q, n))
                q += n
            pos += nreg
        vcalls = []
        q = 0
        while q < CB:
            n = min(CALL_CHUNKS, CB - q)
            vcalls.append((q, n))
            q += n
        gfirst = {g: min(i for i, gg in enumerate(chunk_group) if gg == g)
                  for g in groups}
        glast = {g: max(i for i, gg in enumerate(chunk_group) if gg == g)
                 for g in groups}
        blocks.append(dict(groups=groups, CB=CB, chunk_group=chunk_group,
                           chunk_sub=chunk_sub, seg_start=seg_start,
                           gcalls=gcalls, vcalls=vcalls,
                           gfirst=gfirst, glast=glast,
                           chunk0=tot_chunks))
        tot_chunks += CB
        g0 += GBLK

    CBmax = max(b["CB"] for b in blocks)
    idxw = tot_chunks * 8  # int16 idx columns (128 rows per chunk / 16)

    # per-core static arrays
    per_core = []
    for c in range(ncores):
        m = core_of == c
        es = new_src[m]
        ed_own = new_dst[m] % own
        gg = ed_own // P
        rr = es // SUBR
        # order edges by (g, r) stable; k = running index within (g,r)
        o = np.lexsort((np.arange(es.size), rr, gg))
        es_s, ed_s, gg_s, rr_s = es[o], ed_own[o], gg[o], rr[o]
        key = gg_s * NSUB + rr_s
        firstk = np.r_[True, key[1:] != key[:-1]]
        startk = np.maximum.accumulate(np.where(firstk, np.arange(key.size), 0))
        k = np.arange(key.size) - startk

        srcidx_flat = np.zeros(tot_chunks * P, np.int16)
        dstidx_flat = np.zeros(tot_chunks * P, np.int16)
        dstloc = np.full((P, tot_chunks), 255.0, np.float32)

        # tile chunk for each edge
        blk_of_g = gg_s // GBLK
        seg0 = np.zeros(es_s.size, np.int64)
        chunk0 = np.zeros(es_s.size, np.int64)
        for bi, b in enumerate(blocks):
            mm = blk_of_g == bi
            if not mm.any():
                continue
            ss = np.array([b["seg_start"][(int(g), int(r))]
                           for g, r in zip(gg_s[mm], rr_s[mm])])
            seg0[mm] = ss + b["chunk0"]
        tchunk = seg0 + k // P
        p_e = k % P
        flat = tchunk * P + p_e
        srcidx_flat[flat] = (es_s - rr_s * SUBR).astype(np.int16)
        dstidx_flat[flat] = ed_s.astype(np.int16)
        dstloc.ravel()[p_e * tot_chunks + tchunk] = (ed_s - gg_s * P).astype(
            np.float32)

        # wrapped idx arrays per call (concatenated windows, 8 cols per chunk)
        srcidx16 = np.zeros((P, idxw), np.int16)
        dstidx16 = np.zeros((P, idxw), np.int16)
        for b in blocks:
            c0b = b["chunk0"]
            for (r, q, n) in b["gcalls"]:
                w0 = (c0b + q) * 8
                vals = srcidx_flat[(c0b + q) * P : (c0b + q + n) * P]
                srcidx16[:, w0 : w0 + n * 8] = _interleave16(vals)
            for (q, n) in b["vcalls"]:
                w0 = (c0b + q) * 8
                vals = dstidx_flat[(c0b + q) * P : (c0b + q + n) * P]
                dstidx16[:, w0 : w0 + n * 8] = _interleave16(vals)

        xT_own = np.ascontiguousarray(x[perm[c * own : (c + 1) * own]].T).astype(BF)

        bl = np.full((P, ngroups), 255.0, np.float32)
        for g in range(ngroups):
            size = P if (g < gfull or rem == 0) else rem
            ids = perm[c * own + g * P : c * own + g * P + size]
            bl[:size, g] = batch[ids]

        per_core.append(dict(
            srcidx16=srcidx16, dstidx16=dstidx16,
            dstloc=dstloc.astype(BF), xT_own=xT_own,
            batchloc=bl.astype(BF),
        ))

    meta = dict(N=N, own=own, ngroups=ngroups, gfull=gfull, rem=rem,
                ncores=ncores, blocks=blocks, tot_chunks=tot_chunks,
                CBmax=CBmax, idxw=idxw,
                S_list=[int(v) for v in S_list])
    return per_core, meta, perm


def _gsize(meta, g):
    return P if (g < meta["gfull"] or meta["rem"] == 0) else meta["rem"]


# ---------------------------------------------------------------- kernel build
def _build(meta, debug=False):
    own = meta["own"]
    ngroups = meta["ngroups"]
    ncores = meta["ncores"]
    blocks = meta["blocks"]
    tot_chunks = meta["tot_chunks"]
    CBmax = meta["CBmax"]
    idxw = meta["idxw"]

    nc = bass.Bass(target_bir_lowering=False, debug=True, num_swdge_queues=4)

    # ---- external inputs (per core)
    xT_in = nc.declare_dram_parameter("xT_own", [P, own], BF16, isOutput=False)
    srcidx_in = nc.declare_dram_parameter("srcidx16", [P, idxw], I16, isOutput=False)
    dstidx_in = nc.declare_dram_parameter("dstidx16", [P, idxw], I16, isOutput=False)
    dstloc_in = nc.declare_dram_parameter("dstloc", [P, tot_chunks], BF16,
                                          isOutput=False)
    batchloc_in = nc.declare_dram_parameter("batchloc", [P, ngroups], BF16,
                                            isOutput=False)
    wnames = [
        ("W1_l", [P, D]), ("W1_r", [P, D]), ("W2_l", [P, D]), ("W2_r", [P, D]),
        ("att1_rep", [P, D]), ("att2_rep", [P, D]),
        ("bias1_rep", [P, D]), ("bias2_rep", [P, D]),
        ("b1_l", [1, D]), ("b1_r", [1, D]), ("b2_l", [1, D]), ("b2_r", [1, D]),
        ("iota128", [P, P]), ("iota64", [P, NUM_GRAPHS]),
        ("W3", [P, 10]), ("b3row", [1, 10]), ("ones1", [1, P]),
        ("ones_col", [P, 1]),
    ]
    w_in = {n: nc.declare_dram_parameter(n, sh, BF16, isOutput=False)
            for n, sh in wnames}
    out_t = nc.declare_dram_parameter("out", [NUM_GRAPHS, 10], F32, isOutput=True)
    dbg = {}
    if debug:
        dbg["dbg_h1T"] = nc.declare_dram_parameter("dbg_h1T", [P, own], BF16,
                                                   isOutput=True)

    # ---- internal DRAM
    xl1_own = nc.dram_tensor("xl1_own", [own, D], BF16)
    xr1_own = nc.dram_tensor("xr1_own", [own, D], BF16)
    xl1_full = nc.dram_tensor("xl1_full", [own * ncores, D], BF16,
                              addr_space="Shared")
    xl2_own = nc.dram_tensor("xl2_own", [own, D], BF16)
    xr2_own = nc.dram_tensor("xr2_own", [own, D], BF16)
    xl2_full = nc.dram_tensor("xl2_full", [own * ncores, D], BF16,
                              addr_space="Shared")
    pool_stage = nc.dram_tensor("pool_stage", [NUM_GRAPHS, D + 1], F32)
    pool_red = nc.dram_tensor("pool_red", [NUM_GRAPHS, D + 1], F32,
                              addr_space="Shared")

    # ---- persistent SBUF
    xT_sb = nc.alloc_sbuf_tensor("xT_sb", [P, own], BF16)
    h1T_sb = nc.alloc_sbuf_tensor("h1T_sb", [P, own], BF16)
    dstloc_sb = nc.alloc_sbuf_tensor("dstloc_sb", [P, tot_chunks], BF16)
    batchloc_sb = nc.alloc_sbuf_tensor("batchloc_sb", [P, ngroups], BF16)
    ident_sb = nc.alloc_sbuf_tensor("ident_sb", [P, P], BF16)
    w_sb = {n: nc.alloc_sbuf_tensor(n + "_sb", sh, BF16) for n, sh in wnames}

    nreg = nc.gpsimd.alloc_register("nidx")

    def collective(kind, op, ins, outs):
        nc.gpsimd.collective_compute(
            kind, op, replica_groups=[list(range(ncores))], ins=ins, outs=outs
        )

    # ================= TC-load =================
    with TileContext(nc) as tc:
        with tc.tile_pool(name="idp", bufs=1) as idp:
            idt = idp.tile([P, P], F32)
            make_identity(nc, idt[:])
            nc.vector.tensor_copy(out=ident_sb[:], in_=idt[:])
        nc.gpsimd.load_library(library_config.mlp)
        nc.sync.dma_start(out=xT_sb[:], in_=xT_in[:])
        nc.sync.dma_start(out=dstloc_sb[:], in_=dstloc_in[:])
        nc.sync.dma_start(out=batchloc_sb[:], in_=batchloc_in[:])
        for n, _sh in wnames:
            nc.sync.dma_start(out=w_sb[n][:], in_=w_in[n][:])

    # ================= table build =================
    def build_tables(tc, srcT_sb, Wl, Wr, bl, br, out_l, out_r):
        with (
            tc.tile_pool(name="tp", bufs=3) as tp,
            tc.tile_pool(name="tpp", bufs=3, space="PSUM") as tpp,
        ):
            for g in range(ngroups):
                w = _gsize(meta, g)
                for W, brow, dest in ((Wl, bl, out_l), (Wr, br, out_r)):
                    ps = tpp.tile([P, D], F32, tag="ps")
                    nc.tensor.matmul(out=ps[:w, :],
                                     lhsT=srcT_sb[:, g * P : g * P + w],
                                     rhs=w_sb[W][:], start=True, stop=False)
                    nc.tensor.matmul(out=ps[:w, :], lhsT=w_sb["ones1"][:, :w],
                                     rhs=w_sb[brow][:], start=False, stop=True)
                    ot = tp.tile([P, D], BF16, tag="ot")
                    nc.scalar.activation(out=ot[:w, :], in_=ps[:w, :],
                                         func=mybir.ActivationFunctionType.Copy)
                    nc.sync.dma_start(out=dest[g * P : g * P + w, :],
                                      in_=ot[:w, :])

    # ================= edge layer =================
    def edge_layer(tc, xl_full_t, xr_own_t, att_rep, bias_rep, NH,
                   pool_ctx=None):
        C = D // NH
        Q = D + NH
        qn = [0]

        def next_q():
            qn[0] = (qn[0] + 1) % 4
            return qn[0]

        with (
            tc.tile_pool(name="gp", bufs=3) as gp,
            tc.tile_pool(name="vp", bufs=3) as vp,
            tc.tile_pool(name="rp", bufs=2) as rp,
            tc.tile_pool(name="ip", bufs=3) as ip,
            tc.tile_pool(name="indp", bufs=1) as indp,
            tc.tile_pool(name="sp", bufs=2) as sp,
            tc.tile_pool(name="ep", bufs=2) as ep,
            tc.tile_pool(name="aggp", bufs=2, space="PSUM") as aggp,
            tc.tile_pool(name="tpsum", bufs=2, space="PSUM") as tpsum,
        ):
            for b in blocks:
                CB = b["CB"]
                c0b = b["chunk0"]
                G = gp.tile([P, CBmax * D], BF16, tag="G")
                V = vp.tile([P, CBmax * D], BF16, tag="V")
                gidx = ip.tile([P, CBmax * 8], I16, tag="gidx")
                vidx = ip.tile([P, CBmax * 8], I16, tag="vidx")
                nc.sync.dma_start(out=gidx[:, : CB * 8],
                                  in_=srcidx_in[:, c0b * 8 : (c0b + CB) * 8])
                nc.sync.dma_start(out=vidx[:, : CB * 8],
                                  in_=dstidx_in[:, c0b * 8 : (c0b + CB) * 8])
                # gathers
                for (r, q, n) in b["gcalls"]:
                    nc.gpsimd.reg_mov(nreg, n * P)
                    nc.gpsimd.dma_gather(
                        G[:, q * D : (q + n) * D].rearrange(
                            "p (j c) -> p j c", c=D),
                        xl_full_t[r * SUBR : min((r + 1) * SUBR, own * ncores), :],
                        gidx[:, q * 8 : (q + n) * 8],
                        n * P, nreg, D, single_packet=False, queue_num=next_q())
                for (q, n) in b["vcalls"]:
                    nc.gpsimd.reg_mov(nreg, n * P)
                    nc.gpsimd.dma_gather(
                        V[:, q * D : (q + n) * D].rearrange(
                            "p (j c) -> p j c", c=D),
                        xr_own_t[:],
                        vidx[:, q * 8 : (q + n) * 8],
                        n * P, nreg, D, single_packet=False, queue_num=next_q())
                Gs = G[:, : CB * D]
                Vs = V[:, : CB * D]
                G3 = Gs.rearrange("p (m c) -> p m c", c=D)
                V3 = Vs.rearrange("p (m c) -> p m c", c=D)
                # z = G + V  (into V)
                nc.vector.tensor_tensor(out=V3, in0=G3, in1=V3,
                                        op=mybir.AluOpType.add)
                # leaky in place
                nc.vector.scalar_tensor_tensor(
                    out=Vs, in0=Vs, scalar=NEG_SLOPE, in1=Vs,
                    op0=mybir.AluOpType.mult, op1=mybir.AluOpType.max)
                # m = lr * att
                att_b = w_sb[att_rep][:].unsqueeze(1).broadcast_to([P, CB, D])
                nc.vector.tensor_tensor(out=V3, in0=V3, in1=att_b,
                                        op=mybir.AluOpType.mult)
                # score
                SC = sp.tile([P, CBmax * NH], F32, tag="SC")
                SCs = SC[:, : CB * NH]
                nc.vector.tensor_reduce(
                    out=SCs, in_=Vs.rearrange("p (mh c) -> p mh c", c=C),
                    axis=mybir.AxisListType.X, op=mybir.AluOpType.add)
                # p = exp(score): into PT (for p*G) and into RHS cols D..D+NH
                PT = sp.tile([P, CBmax * NH], BF16, tag="PT")
                PTs = PT[:, : CB * NH]
                nc.scalar.activation(out=PTs, in_=SCs,
                                     func=mybir.ActivationFunctionType.Exp)
                RHS = rp.tile([P, CBmax * Q], BF16, tag="RHS")
                RQ = RHS[:, : CB * Q].rearrange("p (t q) -> p t q", q=Q)
                nc.scalar.activation(
                    out=RQ[:, :, D : D + NH],
                    in_=SCs.rearrange("p (m h) -> p m h", h=NH),
                    func=mybir.ActivationFunctionType.Exp)
                # rhs = G * p
                p_b = PTs.rearrange("p (m h) -> p m h", h=NH).unsqueeze(
                    3).broadcast_to([P, CB, NH, C])
                nc.vector.tensor_tensor(
                    out=RQ[:, :, :D].rearrange("p t (h c) -> p t h c", h=NH),
                    in0=Gs.rearrange("p (m h c) -> p m h c", h=NH, c=C),
                    in1=p_b, op=mybir.AluOpType.mult)
                # indicator
                IND = indp.tile([P, CBmax * D], BF16, tag="IND")
                nc.vector.tensor_tensor(
                    out=IND[:, : CB * D].rearrange("p (t l) -> p t l", l=P),
                    in0=dstloc_sb[:, c0b : c0b + CB].unsqueeze(2).broadcast_to(
                        [P, CB, P]),
                    in1=w_sb["iota128"][:].unsqueeze(1).broadcast_to([P, CB, P]),
                    op=mybir.AluOpType.is_equal)
                # scatter matmuls + per-group epilogue
                aggs = {}
                for g in b["groups"]:
                    agt = aggp.tile([P, Q], F32, tag=f"agg{g % GBLK}",
                                    name=f"agg{g % GBLK}")
                    aggs[g] = agt
                for t in range(CB):
                    g = b["chunk_group"][t]
                    nc.tensor.matmul(
                        out=aggs[g][:],
                        lhsT=IND[:, t * D : (t + 1) * D],
                        rhs=RHS[:, t * Q : (t + 1) * Q],
                        start=(t == b["gfirst"][g]),
                        stop=(t == b["glast"][g]))
                for g in b["groups"]:
                    agg = aggs[g]
                    w = _gsize(meta, g)
                    DEN = ep.tile([P, NH], F32, tag="DEN")
                    nc.vector.tensor_scalar(out=DEN[:], in0=agg[:, D : D + NH],
                                            scalar1=1e-30, scalar2=None,
                                            op0=mybir.AluOpType.max)
                    REC = ep.tile([P, NH], F32, tag="REC")
                    nc.vector.reciprocal(out=REC[:], in_=DEN[:])
                    OUT = ep.tile([P, D], F32, tag="OUT")
                    rec_b = REC[:].unsqueeze(2).broadcast_to([P, NH, C])
                    nc.vector.tensor_tensor(
                        out=OUT[:].rearrange("p (h c) -> p h c", h=NH),
                        in0=agg[:, :D].rearrange("p (h c) -> p h c", h=NH),
                        in1=rec_b, op=mybir.AluOpType.mult)
                    nc.vector.tensor_tensor(out=OUT[:], in0=OUT[:],
                                            in1=w_sb[bias_rep][:],
                                            op=mybir.AluOpType.add)
                    NEG = ep.tile([P, D], F32, tag="NEG")
                    nc.vector.tensor_scalar(out=NEG[:], in0=OUT[:], scalar1=0.0,
                                            scalar2=None,
                                            op0=mybir.AluOpType.min)
                    EN = ep.tile([P, D], F32, tag="EN")
                    nc.scalar.activation(out=EN[:], in_=NEG[:],
                                         func=mybir.ActivationFunctionType.Exp)
                    nc.vector.tensor_scalar(out=OUT[:], in0=OUT[:], scalar1=0.0,
                                            scalar2=None,
                                            op0=mybir.AluOpType.max)
                    H = ep.tile([P, D], BF16, tag="H")
                    nc.vector.scalar_tensor_tensor(
                        out=H[:], in0=EN[:], scalar=-1.0, in1=OUT[:],
                        op0=mybir.AluOpType.add, op1=mybir.AluOpType.add)
                    if pool_ctx is None:
                        tps = tpsum.tile([P, P], BF16, tag="tps")
                        nc.tensor.transpose(out=tps[:, :w], in_=H[:w, :],
                                            identity=ident_sb[:w, :w])
                        nc.scalar.activation(
                            out=h1T_sb[:, g * P : g * P + w], in_=tps[:, :w],
                            func=mybir.ActivationFunctionType.Copy)
                    else:
                        (pool_psum,) = pool_ctx
                        pind = ep.tile([P, NUM_GRAPHS], BF16, tag="pind")
                        nc.vector.tensor_tensor(
                            out=pind[:],
                            in0=batchloc_sb[:, g : g + 1].to_broadcast(
                                [P, NUM_GRAPHS]),
                            in1=w_sb["iota64"][:],
                            op=mybir.AluOpType.is_equal)
                        prhs = ep.tile([P, D + 1], BF16, tag="prhs")
                        nc.vector.tensor_copy(out=prhs[:, :D], in_=H[:])
                        nc.vector.tensor_copy(out=prhs[:, D : D + 1],
                                              in_=w_sb["ones_col"][:])
                        nc.tensor.matmul(out=pool_psum[:], lhsT=pind[:],
                                         rhs=prhs[:],
                                         start=(g == 0),
                                         stop=(g == ngroups - 1))

    # ================= layer 1 =================
    with TileContext(nc) as tc:
        build_tables(tc, xT_sb, "W1_l", "W1_r", "b1_l", "b1_r",
                     xl1_own, xr1_own)

    with TileContext(nc) as tc:
        collective("AllGather", mybir.AluOpType.bypass, [xl1_own[:]],
                   [xl1_full[:]])
        edge_layer(tc, xl1_full, xr1_own, "att1_rep", "bias1_rep", 8)

    # ================= layer 2 =================
    with TileContext(nc) as tc:
        build_tables(tc, h1T_sb, "W2_l", "W2_r", "b2_l", "b2_r",
                     xl2_own, xr2_own)

    with TileContext(nc) as tc:
        collective("AllGather", mybir.AluOpType.bypass, [xl2_own[:]],
                   [xl2_full[:]])
        with tc.tile_pool(name="poolp", bufs=1, space="PSUM") as poolp, \
             tc.tile_pool(name="pstg", bufs=1) as pstg:
            pool_psum = poolp.tile([NUM_GRAPHS, D + 1], F32)
            edge_layer(tc, xl2_full, xr2_own, "att2_rep", "bias2_rep", 1,
                       pool_ctx=(pool_psum,))
            stg = pstg.tile([NUM_GRAPHS, D + 1], F32)
            nc.scalar.activation(out=stg[:], in_=pool_psum[:],
                                 func=mybir.ActivationFunctionType.Copy)
            nc.sync.dma_start(out=pool_stage[:], in_=stg[:])

    # ================= final head =================
    with TileContext(nc) as tc:
        collective("AllReduce", mybir.AluOpType.add, [pool_stage[:]],
                   [pool_red[:]])
        with (
            tc.tile_pool(name="fin", bufs=1) as fin,
            tc.tile_pool(name="finp", bufs=1, space="PSUM") as finp,
        ):
            red = fin.tile([NUM_GRAPHS, D + 1], F32)
            nc.sync.dma_start(out=red[:], in_=pool_red[:])
            cnt = fin.tile([NUM_GRAPHS, 1], F32)
            nc.vector.tensor_scalar(out=cnt[:], in0=red[:, D : D + 1],
                                    scalar1=1.0, scalar2=None,
                                    op0=mybir.AluOpType.max)
            rc = fin.tile([NUM_GRAPHS, 1], F32)
            nc.vector.reciprocal(out=rc[:], in_=cnt[:])
            pooled = fin.tile([NUM_GRAPHS, D], BF16)
            nc.vector.tensor_tensor(out=pooled[:], in0=red[:, :D],
                                    in1=rc[:].to_broadcast([NUM_GRAPHS, D]),
                                    op=mybir.AluOpType.mult)
            tp = finp.tile([P, NUM_GRAPHS], BF16)
            nc.tensor.transpose(out=tp[:], in_=pooled[:],
                                identity=ident_sb[:NUM_GRAPHS, :NUM_GRAPHS])
            pooledT = fin.tile([P, NUM_GRAPHS], BF16)
            nc.scalar.activation(out=pooledT[:], in_=tp[:],
                                 func=mybir.ActivationFunctionType.Copy)
            ops = finp.tile([NUM_GRAPHS, 10], F32)
            nc.tensor.matmul(out=ops[:], lhsT=pooledT[:], rhs=w_sb["W3"][:],
                             start=True, stop=False)
            nc.tensor.matmul(out=ops[:], lhsT=w_sb["ones1"][:, :NUM_GRAPHS],
                             rhs=w_sb["b3row"][:], start=False, stop=True)
            fout = fin.tile([NUM_GRAPHS, 10], F32)
            nc.scalar.activation(out=fout[:], in_=ops[:],
                                 func=mybir.ActivationFunctionType.Copy)
            nc.sync.dma_start(out=out_t[:], in_=fout[:])
        if debug:
            nc.sync.dma_start(out=dbg["dbg_h1T"][:], in_=h1T_sb[:])

    lower_extended_insts(nc)
    _split_waits(nc)
    return nc


# ---------------------------------------------------------------- entry point
def _run(x, edge_index, batch, W1_l, b1_l, W1_r, b1_r, att1, bias1,
         W2_l, b2_l, W2_r, b2_r, att2, bias2, W3, b3, ncores=NCORES,
         debug=False, trace=False):
    x = np.asarray(x, np.float32)
    per_core, meta, perm = _prep(np.asarray(x), np.asarray(edge_index),
                                 np.asarray(batch), ncores)
    consts = dict(
        W1_l=np.asarray(W1_l, np.float32).astype(BF),
        W1_r=np.asarray(W1_r, np.float32).astype(BF),
        W2_l=np.asarray(W2_l, np.float32).astype(BF),
        W2_r=np.asarray(W2_r, np.float32).astype(BF),
        att1_rep=np.tile(np.asarray(att1, np.float32).reshape(1, D),
                         (P, 1)).astype(BF),
        att2_rep=np.tile(np.asarray(att2, np.float32).reshape(1, D),
                         (P, 1)).astype(BF),
        bias1_rep=np.tile(np.asarray(bias1, np.float32).reshape(1, D),
                          (P, 1)).astype(BF),
        bias2_rep=np.tile(np.asarray(bias2, np.float32).reshape(1, D),
                          (P, 1)).astype(BF),
        b1_l=np.asarray(b1_l, np.float32).reshape(1, D).astype(BF),
        b1_r=np.asarray(b1_r, np.float32).reshape(1, D).astype(BF),
        b2_l=np.asarray(b2_l, np.float32).reshape(1, D).astype(BF),
        b2_r=np.asarray(b2_r, np.float32).reshape(1, D).astype(BF),
        iota128=np.tile(np.arange(P, dtype=np.float32).reshape(1, P),
                        (P, 1)).astype(BF),
        iota64=np.tile(np.arange(NUM_GRAPHS, dtype=np.float32).reshape(
            1, NUM_GRAPHS), (P, 1)).astype(BF),
        W3=np.asarray(W3, np.float32).astype(BF),
        b3row=np.asarray(b3, np.float32).reshape(1, 10).astype(BF),
        ones1=np.ones((1, P), np.float32).astype(BF),
        ones_col=np.ones((P, 1), np.float32).astype(BF),
    )
    nc = _build(meta, debug=debug)
    in_maps = []
    for c in range(ncores):
        m = dict(per_core[c])
        m.update(consts)
        in_maps.append(m)
    if trace:
        _install_profhook()
    res = run_bass_kernel_spmd(nc, in_maps, core_ids=list(range(ncores)),
                               trace=trace)
    return res.results[0]["out"].astype(np.float32), (res, per_core, meta, perm)


def kernel(**inputs):
    out, _res = _run(**inputs)
    return out


# revision 9
# speedup vs baseline: 1.7559x; 1.0166x over previous
"""2-layer GATv2 + global mean pool + linear head, on 8 Trainium2 NeuronCores.

Strategy (dst-sharded, degree-balanced, batched dma_gather):
  - Nodes are relabeled by a degree-balanced bin-packing into groups of <=128
    destination nodes with near-equal incident-edge counts.  Core c owns
    nodes [c*own, (c+1)*own); edges are assigned by destination owner.
  - Edges live in a static chunk grid (128 edge slots per chunk).  Each
    group's chunks are split into 4 source-subrange segments (int16
    dma_gather indices address <=32768 table rows) with per-(group,subrange)
    chunk budgets shared across cores, so one SPMD program fits all cores.
  - Blocks of 3 groups are processed at a time: batched dma_gather calls
    (<=4 chunks each, 4 SWDGE queues) fetch xl[src] (G) from the AllGather'd
    table and xr[dst] (V) from the core-local table; all elementwise work
    (z=G+V, leaky, att-mult, segment-score reduce, exp, p*G) runs as a few
    block-wide vector/ACT ops; a per-chunk PE matmul with the is_equal
    indicator scatters [p*G | p] into per-group PSUM accumulators (f32),
    giving numerators and softmax denominators at once.
  - Layer tables xl/xr = x@W + b are built on device; xl is AllGather'ed.
    Final pooling partial sums are AllReduce'd; every core computes the
    tiny output head.
"""

import sys

for _p in ("/opt/trn_rl_repo",):
    if _p not in sys.path:
        sys.path.insert(0, _p)

import numpy as np
import ml_dtypes

BF = ml_dtypes.bfloat16

import concourse.bass as bass
import concourse.mybir as mybir
from concourse.tile import TileContext
from concourse.bass_utils import run_bass_kernel_spmd
from concourse.masks import make_identity
from concourse import library_config
from concourse.library_overlay import lower_extended_insts

F32 = mybir.dt.float32
BF16 = mybir.dt.bfloat16
I16 = mybir.dt.int16
P = 128
D = 128
NCORES = 8
NUM_GRAPHS = 64
NEG_SLOPE = 0.2
SUBR = 25000          # int16 dma_gather window (4 x 25000 = 100000 rows)
NSUB = 4
GBLK = 2              # groups per processing block
CALL_CHUNKS = 8       # chunks per dma_gather call (1024 rows)


# ---------------------------------------------------------------- prof hook
def _install_profhook():
    """Provide antenv.axon_hooks (absent in this image) so trace=True works."""
    import types

    if "antenv.axon_hooks" in sys.modules:
        return
    try:
        from trn_agent_boot.trn_boot import _ntff_profile_via_ctypes
    except Exception:
        return
    mod = types.ModuleType("antenv.axon_hooks")
    mod._hook = None
    mod.set_axon_ntff_profile_hook = lambda h: setattr(mod, "_hook", h)
    mod.get_axon_ntff_profile_hook = lambda: mod._hook
    sys.modules["antenv.axon_hooks"] = mod
    try:
        mod._hook = _ntff_profile_via_ctypes("/opt/axon/libaxon_pjrt.so")
    except Exception:
        mod._hook = None


# ---------------------------------------------------------------- wait split
def _split_waits(nc, max_waits=1):
    """walrus TPB_CTRL codegen rejects >1 sync-wait per instruction; move
    extras onto preceding NoOps on the same engine."""
    n_added = 0
    for fn in nc.m.functions:
        for blk in fn.blocks:
            new_insts = []
            for inst in blk.instructions:
                si = getattr(inst, "sync_info", None)
                waits = list(si.on_wait) if si is not None and si.on_wait else []
                if len(waits) > max_waits:
                    extra = waits[:-max_waits]
                    for i in range(0, len(extra), max_waits):
                        chunk = extra[i : i + max_waits]
                        nop = mybir.InstNoOp(
                            name=f"{inst.name}_wsplit{n_added}",
                            engine=inst.engine,
                            ins=[],
                            outs=[],
                            sync_info=mybir.SyncInfo(on_wait=chunk, on_update=[]),
                        )
                        n_added += 1
                        new_insts.append(nop)
                    si.on_wait = waits[-max_waits:]
                new_insts.append(inst)
            blk.instructions = new_insts
    return n_added


# ---------------------------------------------------------------- host prep
def _interleave16(vals):
    """[n] int16 -> [128, n/16] wrapped (i at [i%16, i//16]) and tiled x8."""
    n = vals.size
    a = np.zeros((16, n // 16), np.int16)
    a[np.arange(n) % 16, np.arange(n) // 16] = vals
    return np.tile(a, (8, 1))


def _prep(x, edge_index, batch, ncores):
    """Degree-balanced relabeling + blocked subrange chunk schedule."""
    N = x.shape[0]
    own = N // ncores
    gfull, rem = divmod(own, P)
    ngroups = gfull + (1 if rem else 0)

    src = np.concatenate([edge_index[0].astype(np.int64), np.arange(N)])
    dst = np.concatenate([edge_index[1].astype(np.int64), np.arange(N)])
    deg = np.bincount(dst, minlength=N)

    # degree-balanced bin packing (equal edge counts per group, all cores)
    caps = []
    for c in range(ncores):
        caps += [P] * gfull + ([rem] if rem else [])
    nbins = len(caps)
    import heapq

    heap = [(0, b) for b in range(nbins)]
    heapq.heapify(heap)
    bin_nodes = [[] for _ in range(nbins)]
    order = np.argsort(-deg, kind="stable")
    for node in order:
        while True:
            s, b = heapq.heappop(heap)
            if len(bin_nodes[b]) < caps[b]:
                break
        bin_nodes[b].append(node)
        if len(bin_nodes[b]) < caps[b]:
            heapq.heappush(heap, (s + int(deg[node]), b))

    perm = np.empty(N, np.int64)  # perm[new] = old
    for b in range(nbins):
        c, g = divmod(b, ngroups)
        base = c * own + g * P
        nodes = bin_nodes[b]
        perm[base : base + len(nodes)] = nodes
    inv = np.empty(N, np.int64)
    inv[perm] = np.arange(N)

    new_src = inv[src]
    new_dst = inv[dst]
    core_of = new_dst // own

    # per (core, group, subrange) edge lists and counts
    sub_of = new_src // SUBR
    n_cgr = np.zeros((ncores, ngroups, NSUB), np.int64)
    for c in range(ncores):
        m = core_of == c
        gg = (new_dst[m] % own) // P
        rr = sub_of[m]
        np.add.at(n_cgr[c], (gg, rr), 1)
    B = np.ceil(n_cgr.max(axis=0) / P).astype(np.int64)  # [ngroups, NSUB]
    S_list = B.sum(axis=1)  # chunks per group

    # block schedule (shared across cores)
    blocks = []
    tot_chunks = 0
    g0 = 0
    while g0 < ngroups:
        groups = list(range(g0, min(g0 + GBLK, ngroups)))
        chunk_group = []       # tile-chunk -> group
        chunk_sub = []         # tile-chunk -> subrange
        seg_start = {}         # (g, r) -> first tile-chunk (block-local)
        for r in range(NSUB):
            for g in groups:
                seg_start[(g, r)] = len(chunk_group)
                chunk_group += [g] * int(B[g, r])
                chunk_sub += [r] * int(B[g, r])
        CB = len(chunk_group)
        # gather calls: per r-region, sub-calls of <= CALL_CHUNKS chunks
        gcalls = []
        pos = 0
        for r in range(NSUB):
            nreg = int(B[groups, r].sum() if hasattr(B, "sum") else 0)
            nreg = int(sum(int(B[g, r]) for g in groups))
            q = 0
            while q < nreg:
                n = min(CALL_CHUNKS, nreg - q)
                gcalls.append((r, pos + q, n))
                q += n
            pos += nreg
        vcalls = []
        q = 0
        while q < CB:
            n = min(CALL_CHUNKS, CB - q)
            vcalls.append((q, n))
            q += n
        gfirst = {g: min(i for i, gg in enumerate(chunk_group) if gg == g)
                  for g in groups}
        glast = {g: max(i for i, gg in enumerate(chunk_group) if gg == g)
                 for g in groups}
        blocks.append(dict(groups=groups, CB=CB, chunk_group=chunk_group,
                           chunk_sub=chunk_sub, seg_start=seg_start,
                           gcalls=gcalls, vcalls=vcalls,
                           gfirst=gfirst, glast=glast,
                           chunk0=tot_chunks))
        tot_chunks += CB
        g0 += GBLK

    CBmax = max(b["CB"] for b in blocks)
    idxw = tot_chunks * 8  # int16 idx columns (128 rows per chunk / 16)

    # per-core static arrays
    per_core = []
    for c in range(ncores):
        m = core_of == c
        es = new_src[m]
        ed_own = new_dst[m] % own
        gg = ed_own // P
        rr = es // SUBR
        # order edges by (g, r) stable; k = running index within (g,r)
        o = np.lexsort((np.arange(es.size), rr, gg))
        es_s, ed_s, gg_s, rr_s = es[o], ed_own[o], gg[o], rr[o]
        key = gg_s * NSUB + rr_s
        firstk = np.r_[True, key[1:] != key[:-1]]
        startk = np.maximum.accumulate(np.where(firstk, np.arange(key.size), 0))
        k = np.arange(key.size) - startk

        srcidx_flat = np.zeros(tot_chunks * P, np.int16)
        dstidx_flat = np.zeros(tot_chunks * P, np.int16)
        dstloc = np.full((P, tot_chunks), 255.0, np.float32)

        # tile chunk for each edge
        blk_of_g = gg_s // GBLK
        seg0 = np.zeros(es_s.size, np.int64)
        chunk0 = np.zeros(es_s.size, np.int64)
        for bi, b in enumerate(blocks):
            mm = blk_of_g == bi
            if not mm.any():
                continue
            ss = np.array([b["seg_start"][(int(g), int(r))]
                           for g, r in zip(gg_s[mm], rr_s[mm])])
            seg0[mm] = ss + b["chunk0"]
        tchunk = seg0 + k // P
        p_e = k % P
        flat = tchunk * P + p_e
        srcidx_flat[flat] = (es_s - rr_s * SUBR).astype(np.int16)
        dstidx_flat[flat] = ed_s.astype(np.int16)
        dstloc.ravel()[p_e * tot_chunks + tchunk] = (ed_s - gg_s * P).astype(
            np.float32)

        # wrapped idx arrays per call (concatenated windows, 8 cols per chunk)
        srcidx16 = np.zeros((P, idxw), np.int16)
        dstidx16 = np.zeros((P, idxw), np.int16)
        for b in blocks:
            c0b = b["chunk0"]
            for (r, q, n) in b["gcalls"]:
                w0 = (c0b + q) * 8
                vals = srcidx_flat[(c0b + q) * P : (c0b + q + n) * P]
                srcidx16[:, w0 : w0 + n * 8] = _interleave16(vals)
            for (q, n) in b["vcalls"]:
                w0 = (c0b + q) * 8
                vals = dstidx_flat[(c0b + q) * P : (c0b + q + n) * P]
                dstidx16[:, w0 : w0 + n * 8] = _interleave16(vals)

        xT_own = np.ascontiguousarray(x[perm[c * own : (c + 1) * own]].T).astype(BF)

        bl = np.full((P, ngroups), 255.0, np.float32)
        for g in range(ngroups):
            size = P if (g < gfull or rem == 0) else rem
            ids = perm[c * own + g * P : c * own + g * P + size]
            bl[:size, g] = batch[ids]

        per_core.append(dict(
            srcidx16=srcidx16, dstidx16=dstidx16,
            dstloc=dstloc.astype(BF), xT_own=xT_own,
            batchloc=bl.astype(BF),
        ))

    meta = dict(N=N, own=own, ngroups=ngroups, gfull=gfull, rem=rem,
                ncores=ncores, blocks=blocks, tot_chunks=tot_chunks,
                CBmax=CBmax, idxw=idxw,
                S_list=[int(v) for v in S_list])
    return per_core, meta, perm


def _gsize(meta, g):
    return P if (g < meta["gfull"] or meta["rem"] == 0) else meta["rem"]


# ---------------------------------------------------------------- kernel build
def _build(meta, debug=False):
    own = meta["own"]
    ngroups = meta["ngroups"]
    ncores = meta["ncores"]
    blocks = meta["blocks"]
    tot_chunks = meta["tot_chunks"]
    CBmax = meta["CBmax"]
    idxw = meta["idxw"]

    nc = bass.Bass(target_bir_lowering=False, debug=True, num_swdge_queues=4)

    # ---- external inputs (per core)
    xT_in = nc.declare_dram_parameter("xT_own", [P, own], BF16, isOutput=False)
    srcidx_in = nc.declare_dram_parameter("srcidx16", [P, idxw], I16, isOutput=False)
    dstidx_in = nc.declare_dram_parameter("dstidx16", [P, idxw], I16, isOutput=False)
    dstloc_in = nc.declare_dram_parameter("dstloc", [P, tot_chunks], BF16,
                                          isOutput=False)
    batchloc_in = nc.declare_dram_parameter("batchloc", [P, ngroups], BF16,
                                            isOutput=False)
    wnames = [
        ("W1_l", [P, D]), ("W1_r", [P, D]), ("W2_l", [P, D]), ("W2_r", [P, D]),
        ("att1_rep", [P, D]), ("att2_rep", [P, D]),
        ("bias1_rep", [P, D]), ("bias2_rep", [P, D]),
        ("b1_l", [1, D]), ("b1_r", [1, D]), ("b2_l", [1, D]), ("b2_r", [1, D]),
        ("iota128", [P, P]), ("iota64", [P, NUM_GRAPHS]),
        ("W3", [P, 10]), ("b3row", [1, 10]), ("ones1", [1, P]),
        ("ones_col", [P, 1]),
    ]
    w_in = {n: nc.declare_dram_parameter(n, sh, BF16, isOutput=False)
            for n, sh in wnames}
    out_t = nc.declare_dram_parameter("out", [NUM_GRAPHS, 10], F32, isOutput=True)
    dbg = {}
    if debug:
        dbg["dbg_h1T"] = nc.declare_dram_parameter("dbg_h1T", [P, own], BF16,
                                                   isOutput=True)

    # ---- internal DRAM
    xl1_own = nc.dram_tensor("xl1_own", [own, D], BF16)
    xr1_own = nc.dram_tensor("xr1_own", [own, D], BF16)
    xl1_full = nc.dram_tensor("xl1_full", [own * ncores, D], BF16,
                              addr_space="Shared")
    xl2_own = nc.dram_tensor("xl2_own", [own, D], BF16)
    xr2_own = nc.dram_tensor("xr2_own", [own, D], BF16)
    xl2_full = nc.dram_tensor("xl2_full", [own * ncores, D], BF16,
                              addr_space="Shared")
    pool_stage = nc.dram_tensor("pool_stage", [NUM_GRAPHS, D + 1], F32)
    pool_red = nc.dram_tensor("pool_red", [NUM_GRAPHS, D + 1], F32,
                              addr_space="Shared")

    # ---- persistent SBUF
    xT_sb = nc.alloc_sbuf_tensor("xT_sb", [P, own], BF16)
    h1T_sb = nc.alloc_sbuf_tensor("h1T_sb", [P, own], BF16)
    dstloc_sb = nc.alloc_sbuf_tensor("dstloc_sb", [P, tot_chunks], BF16)
    batchloc_sb = nc.alloc_sbuf_tensor("batchloc_sb", [P, ngroups], BF16)
    ident_sb = nc.alloc_sbuf_tensor("ident_sb", [P, P], BF16)
    w_sb = {n: nc.alloc_sbuf_tensor(n + "_sb", sh, BF16) for n, sh in wnames}

    nreg = nc.gpsimd.alloc_register("nidx")

    def collective(kind, op, ins, outs):
        nc.gpsimd.collective_compute(
            kind, op, replica_groups=[list(range(ncores))], ins=ins, outs=outs
        )

    # ================= TC-load =================
    with TileContext(nc) as tc:
        with tc.tile_pool(name="idp", bufs=1) as idp:
            idt = idp.tile([P, P], F32)
            make_identity(nc, idt[:])
            nc.vector.tensor_copy(out=ident_sb[:], in_=idt[:])
        nc.gpsimd.load_library(library_config.mlp)
        nc.sync.dma_start(out=xT_sb[:], in_=xT_in[:])
        nc.sync.dma_start(out=dstloc_sb[:], in_=dstloc_in[:])
        nc.sync.dma_start(out=batchloc_sb[:], in_=batchloc_in[:])
        for n, _sh in wnames:
            nc.sync.dma_start(out=w_sb[n][:], in_=w_in[n][:])

    # ================= table build =================
    def build_tables(tc, srcT_sb, Wl, Wr, bl, br, out_l, out_r):
        with (
            tc.tile_pool(name="tp", bufs=3) as tp,
            tc.tile_pool(name="tpp", bufs=3, space="PSUM") as tpp,
        ):
            for g in range(ngroups):
                w = _gsize(meta, g)
                for W, brow, dest in ((Wl, bl, out_l), (Wr, br, out_r)):
                    ps = tpp.tile([P, D], F32, tag="ps")
                    nc.tensor.matmul(out=ps[:w, :],
                                     lhsT=srcT_sb[:, g * P : g * P + w],
                                     rhs=w_sb[W][:], start=True, stop=False)
                    nc.tensor.matmul(out=ps[:w, :], lhsT=w_sb["ones1"][:, :w],
                                     rhs=w_sb[brow][:], start=False, stop=True)
                    ot = tp.tile([P, D], BF16, tag="ot")
                    nc.scalar.activation(out=ot[:w, :], in_=ps[:w, :],
                                         func=mybir.ActivationFunctionType.Copy)
                    nc.sync.dma_start(out=dest[g * P : g * P + w, :],
                                      in_=ot[:w, :])

    # ================= edge layer =================
    def edge_layer(tc, xl_full_t, xr_own_t, att_rep, bias_rep, NH,
                   pool_ctx=None):
        C = D // NH
        Q = D + NH
        qn = [0]

        def next_q():
            qn[0] = (qn[0] + 1) % 4
            return qn[0]

        with (
            tc.tile_pool(name="gp", bufs=3) as gp,
            tc.tile_pool(name="vp", bufs=3) as vp,
            tc.tile_pool(name="rp", bufs=2) as rp,
            tc.tile_pool(name="ip", bufs=3) as ip,
            tc.tile_pool(name="indp", bufs=1) as indp,
            tc.tile_pool(name="sp", bufs=2) as sp,
            tc.tile_pool(name="ep", bufs=2) as ep,
            tc.tile_pool(name="aggp", bufs=2, space="PSUM") as aggp,
            tc.tile_pool(name="tpsum", bufs=2, space="PSUM") as tpsum,
        ):
            for b in blocks:
                CB = b["CB"]
                c0b = b["chunk0"]
                G = gp.tile([P, CBmax * D], BF16, tag="G")
                V = vp.tile([P, CBmax * D], BF16, tag="V")
                gidx = ip.tile([P, CBmax * 8], I16, tag="gidx")
                vidx = ip.tile([P, CBmax * 8], I16, tag="vidx")
                nc.sync.dma_start(out=gidx[:, : CB * 8],
                                  in_=srcidx_in[:, c0b * 8 : (c0b + CB) * 8])
                nc.sync.dma_start(out=vidx[:, : CB * 8],
                                  in_=dstidx_in[:, c0b * 8 : (c0b + CB) * 8])
                # gathers
                for (r, q, n) in b["gcalls"]:
                    nc.gpsimd.reg_mov(nreg, n * P)
                    nc.gpsimd.dma_gather(
                        G[:, q * D : (q + n) * D].rearrange(
                            "p (j c) -> p j c", c=D),
                        xl_full_t[r * SUBR : min((r + 1) * SUBR, own * ncores), :],
                        gidx[:, q * 8 : (q + n) * 8],
                        n * P, nreg, D, single_packet=False, queue_num=next_q())
                for (q, n) in b["vcalls"]:
                    nc.gpsimd.reg_mov(nreg, n * P)
                    nc.gpsimd.dma_gather(
                        V[:, q * D : (q + n) * D].rearrange(
                            "p (j c) -> p j c", c=D),
                        xr_own_t[:],
                        vidx[:, q * 8 : (q + n) * 8],
                        n * P, nreg, D, single_packet=False, queue_num=next_q())
                Gs = G[:, : CB * D]
                Vs = V[:, : CB * D]
                G3 = Gs.rearrange("p (m c) -> p m c", c=D)
                V3 = Vs.rearrange("p (m c) -> p m c", c=D)
                # z = G + V  (into V)
                nc.vector.tensor_tensor(out=V3, in0=G3, in1=V3,
                                        op=mybir.AluOpType.add)
                # leaky in place
                nc.vector.scalar_tensor_tensor(
                    out=Vs, in0=Vs, scalar=NEG_SLOPE, in1=Vs,
                    op0=mybir.AluOpType.mult, op1=mybir.AluOpType.max)
                # m = lr * att
                att_b = w_sb[att_rep][:].unsqueeze(1).broadcast_to([P, CB, D])
                nc.vector.tensor_tensor(out=V3, in0=V3, in1=att_b,
                                        op=mybir.AluOpType.mult)
                # score
                SC = sp.tile([P, CBmax * NH], F32, tag="SC")
                SCs = SC[:, : CB * NH]
                nc.vector.tensor_reduce(
                    out=SCs, in_=Vs.rearrange("p (mh c) -> p mh c", c=C),
                    axis=mybir.AxisListType.X, op=mybir.AluOpType.add)
                # p = exp(score): into PT (for p*G) and into RHS cols D..D+NH
                PT = sp.tile([P, CBmax * NH], BF16, tag="PT")
                PTs = PT[:, : CB * NH]
                nc.scalar.activation(out=PTs, in_=SCs,
                                     func=mybir.ActivationFunctionType.Exp)
                RHS = rp.tile([P, CBmax * Q], BF16, tag="RHS")
                RQ = RHS[:, : CB * Q].rearrange("p (t q) -> p t q", q=Q)
                nc.scalar.activation(
                    out=RQ[:, :, D : D + NH],
                    in_=SCs.rearrange("p (m h) -> p m h", h=NH),
                    func=mybir.ActivationFunctionType.Exp)
                # rhs = G * p
                p_b = PTs.rearrange("p (m h) -> p m h", h=NH).unsqueeze(
                    3).broadcast_to([P, CB, NH, C])
                nc.vector.tensor_tensor(
                    out=RQ[:, :, :D].rearrange("p t (h c) -> p t h c", h=NH),
                    in0=Gs.rearrange("p (m h c) -> p m h c", h=NH, c=C),
                    in1=p_b, op=mybir.AluOpType.mult)
                # indicator
                IND = indp.tile([P, CBmax * D], BF16, tag="IND")
                nc.vector.tensor_tensor(
                    out=IND[:, : CB * D].rearrange("p (t l) -> p t l", l=P),
                    in0=dstloc_sb[:, c0b : c0b + CB].unsqueeze(2).broadcast_to(
                        [P, CB, P]),
                    in1=w_sb["iota128"][:].unsqueeze(1).broadcast_to([P, CB, P]),
                    op=mybir.AluOpType.is_equal)
                # scatter matmuls + per-group epilogue
                aggs = {}
                for g in b["groups"]:
                    agt = aggp.tile([P, Q], F32, tag=f"agg{g % GBLK}",
                                    name=f"agg{g % GBLK}")
                    aggs[g] = agt
                for t in range(CB):
                    g = b["chunk_group"][t]
                    nc.tensor.matmul(
                        out=aggs[g][:],
                        lhsT=IND[:, t * D : (t + 1) * D],
                        rhs=RHS[:, t * Q : (t + 1) * Q],
                        start=(t == b["gfirst"][g]),
                        stop=(t == b["glast"][g]))
                for g in b["groups"]:
                    agg = aggs[g]
                    w = _gsize(meta, g)
                    DEN = ep.tile([P, NH], F32, tag="DEN")
                    nc.vector.tensor_scalar(out=DEN[:], in0=agg[:, D : D + NH],
                                            scalar1=1e-30, scalar2=None,
                                            op0=mybir.AluOpType.max)
                    REC = ep.tile([P, NH], F32, tag="REC")
                    nc.vector.reciprocal(out=REC[:], in_=DEN[:])
                    OUT = ep.tile([P, D], F32, tag="OUT")
                    rec_b = REC[:].unsqueeze(2).broadcast_to([P, NH, C])
                    nc.vector.tensor_tensor(
                        out=OUT[:].rearrange("p (h c) -> p h c", h=NH),
                        in0=agg[:, :D].rearrange("p (h c) -> p h c", h=NH),
                        in1=rec_b, op=mybir.AluOpType.mult)
                    nc.vector.tensor_tensor(out=OUT[:], in0=OUT[:],
                                            in1=w_sb[bias_rep][:],
                                            op=mybir.AluOpType.add)
                    NEG = ep.tile([P, D], F32, tag="NEG")
                    nc.vector.tensor_scalar(out=NEG[:], in0=OUT[:], scalar1=0.0,
                                            scalar2=None,
                                            op0=mybir.AluOpType.min)
                    EN = ep.tile([P, D], F32, tag="EN")
                    nc.scalar.activation(out=EN[:], in_=NEG[:],
                                         func=mybir.ActivationFunctionType.Exp)
                    nc.vector.tensor_scalar(out=OUT[:], in0=OUT[:], scalar1=0.0,
                                            scalar2=None,
                                            op0=mybir.AluOpType.max)
                    H = ep.tile([P, D], BF16, tag="H")
                    nc.vector.scalar_tensor_tensor(
                        out=H[:], in0=EN[:], scalar=-1.0, in1=OUT[:],
                        op0=mybir.AluOpType.add, op1=mybir.AluOpType.add)
                    if pool_ctx is None:
                        tps = tpsum.tile([P, P], BF16, tag="tps")
                        nc.tensor.transpose(out=tps[:, :w], in_=H[:w, :],
                                            identity=ident_sb[:w, :w])
                        nc.scalar.activation(
                            out=h1T_sb[:, g * P : g * P + w], in_=tps[:, :w],
                            func=mybir.ActivationFunctionType.Copy)
                    else:
                        (pool_psum,) = pool_ctx
                        pind = ep.tile([P, NUM_GRAPHS], BF16, tag="pind")
                        nc.vector.tensor_tensor(
                            out=pind[:],
                            in0=batchloc_sb[:, g : g + 1].to_broadcast(
                                [P, NUM_GRAPHS]),
                            in1=w_sb["iota64"][:],
                            op=mybir.AluOpType.is_equal)
                        prhs = ep.tile([P, D + 1], BF16, tag="prhs")
                        nc.vector.tensor_copy(out=prhs[:, :D], in_=H[:])
                        nc.vector.tensor_copy(out=prhs[:, D : D + 1],
                                              in_=w_sb["ones_col"][:])
                        nc.tensor.matmul(out=pool_psum[:], lhsT=pind[:],
                                         rhs=prhs[:],
                                         start=(g == 0),
                                         stop=(g == ngroups - 1))

    # ================= layer 1 =================
    with TileContext(nc) as tc:
        build_tables(tc, xT_sb, "W1_l", "W1_r", "b1_l", "b1_r",
                     xl1_own, xr1_own)

    with TileContext(nc) as tc:
        collective("AllGather", mybir.AluOpType.bypass, [xl1_own[:]],
                   [xl1_full[:]])
        edge_layer(tc, xl1_full, xr1_own, "att1_rep", "bias1_rep", 8)

    # ================= layer 2 =================
    with TileContext(nc) as tc:
        build_tables(tc, h1T_sb, "W2_l", "W2_r", "b2_l", "b2_r",
                     xl2_own, xr2_own)

    with TileContext(nc) as tc:
        collective("AllGather", mybir.AluOpType.bypass, [xl2_own[:]],
                   [xl2_full[:]])
        with tc.tile_pool(name="poolp", bufs=1, space="PSUM") as poolp, \
             tc.tile_pool(name="pstg", bufs=1) as pstg:
            pool_psum = poolp.tile([NUM_GRAPHS, D + 1], F32)
            edge_layer(tc, xl2_full, xr2_own, "att2_rep", "bias2_rep", 1,
                       pool_ctx=(pool_psum,))
            stg = pstg.tile([NUM_GRAPHS, D + 1], F32)
            nc.scalar.activation(out=stg[:], in_=pool_psum[:],
                                 func=mybir.ActivationFunctionType.Copy)
            nc.sync.dma_start(out=pool_stage[:], in_=stg[:])

    # ================= final head =================
    with TileContext(nc) as tc:
        collective("AllReduce", mybir.AluOpType.add, [pool_stage[:]],
                   [pool_red[:]])
        with (
            tc.tile_pool(name="fin", bufs=1) as fin,
            tc.tile_pool(name="finp", bufs=1, space="PSUM") as finp,
        ):
            red = fin.tile([NUM_GRAPHS, D + 1], F32)
            nc.sync.dma_start(out=red[:], in_=pool_red[:])
            cnt = fin.tile([NUM_GRAPHS, 1], F32)
            nc.vector.tensor_scalar(out=cnt[:], in0=red[:, D : D + 1],
                                    scalar1=1.0, scalar2=None,
                                    op0=mybir.AluOpType.max)
            rc = fin.tile([NUM_GRAPHS, 1], F32)
            nc.vector.reciprocal(out=rc[:], in_=cnt[:])
            pooled = fin.tile([NUM_GRAPHS, D], BF16)
            nc.vector.tensor_tensor(out=pooled[:], in0=red[:, :D],
                                    in1=rc[:].to_broadcast([NUM_GRAPHS, D]),
                                    op=mybir.AluOpType.mult)
            tp = finp.tile([P, NUM_GRAPHS], BF16)
            nc.tensor.transpose(out=tp[:], in_=pooled[:],
                                identity=ident_sb[:NUM_GRAPHS, :NUM_GRAPHS])
            pooledT = fin.tile([P, NUM_GRAPHS], BF16)
            nc.scalar.activation(out=pooledT[:], in_=tp[:],
                                 func=mybir.ActivationFunctionType.Copy)
            ops = finp.tile([NUM_GRAPHS, 10], F32)
            nc.tensor.matmul(out=ops[:], lhsT=pooledT[:], rhs=w_sb["W3"][:],
                             start=True, stop=False)
            nc.tensor.matmul(out=ops[:], lhsT=w_sb["ones1"][:, :NUM_GRAPHS],
                             rhs=w_sb["b3row"][:], start=False, stop=True)
            fout = fin.tile([NUM_GRAPHS, 10], F32)
            nc.scalar.activation(out=fout[:], in_=ops[:],
                                 func=mybir.ActivationFunctionType.Copy)
            nc.sync.dma_start(out=out_t[:], in_=fout[:])
        if debug:
            nc.sync.dma_start(out=dbg["dbg_h1T"][:], in_=h1T_sb[:])

    lower_extended_insts(nc)
    _split_waits(nc)
    return nc


# ---------------------------------------------------------------- entry point
def _run(x, edge_index, batch, W1_l, b1_l, W1_r, b1_r, att1, bias1,
         W2_l, b2_l, W2_r, b2_r, att2, bias2, W3, b3, ncores=NCORES,
         debug=False, trace=False):
    x = np.asarray(x, np.float32)
    per_core, meta, perm = _prep(np.asarray(x), np.asarray(edge_index),
                                 np.asarray(batch), ncores)
    consts = dict(
        W1_l=np.asarray(W1_l, np.float32).astype(BF),
        W1_r=np.asarray(W1_r, np.float32).astype(BF),
        W2_l=np.asarray(W2_l, np.float32).astype(BF),
        W2_r=np.asarray(W2_r, np.float32).astype(BF),
        att1_rep=np.tile(np.asarray(att1, np.float32).reshape(1, D),
                         (P, 1)).astype(BF),
        att2_rep=np.tile(np.asarray(att2, np.float32).reshape(1, D),
                         (P, 1)).astype(BF),
        bias1_rep=np.tile(np.asarray(bias1, np.float32).reshape(1, D),
                          (P, 1)).astype(BF),
        bias2_rep=np.tile(np.asarray(bias2, np.float32).reshape(1, D),
                          (P, 1)).astype(BF),
        b1_l=np.asarray(b1_l, np.float32).reshape(1, D).astype(BF),
        b1_r=np.asarray(b1_r, np.float32).reshape(1, D).astype(BF),
        b2_l=np.asarray(b2_l, np.float32).reshape(1, D).astype(BF),
        b2_r=np.asarray(b2_r, np.float32).reshape(1, D).astype(BF),
        iota128=np.tile(np.arange(P, dtype=np.float32).reshape(1, P),
                        (P, 1)).astype(BF),
        iota64=np.tile(np.arange(NUM_GRAPHS, dtype=np.float32).reshape(
            1, NUM_GRAPHS), (P, 1)).astype(BF),
        W3=np.asarray(W3, np.float32).astype(BF),
        b3row=np.asarray(b3, np.float32).reshape(1, 10).astype(BF),
        ones1=np.ones((1, P), np.float32).astype(BF),
        ones_col=np.ones((P, 1), np.float32).astype(BF),
    )
    nc = _build(meta, debug=debug)
    in_maps = []
    for c in range(ncores):
        m = dict(per_core[c])
        m.update(consts)
        in_maps.append(m)
    if trace:
        _install_profhook()
    res = run_bass_kernel_spmd(nc, in_maps, core_ids=list(range(ncores)),
                               trace=trace)
    return res.results[0]["out"].astype(np.float32), (res, per_core, meta, perm)


def kernel(**inputs):
    out, _res = _run(**inputs)
    return out
